# revision 1
# baseline (speedup 1.0000x reference)
"""Trainium2 Bass kernel for the sparse_attention nn.Module problem.

Strategy: data-parallel over the MSA-row dim S (S=128 -> 16 rows per core,
8 cores). All projection weights + pair bias replicated; mask bias and
activations sharded with S. No collectives.

Per-core dataflow (layouts chosen so no on-device input transposes are
needed; host pre-transposes x to [s, c, q] and pre-exponentiates the pair
bias). Matmul chain runs in fp16 (attention weights bf16 for the e^±60
dynamic range); PSUM accumulation is fp32 throughout:
  qT/kT = W @ x^T            (PSUM fp32, DVE evict to fp16; then DMA-remap
                              to a head-flat [d,(h,tc,q)] layout at partition
                              base 0 -- the PE cannot mix tile positions)
  v     = kv_x @ Wv^T        (natural [k, t] layout, evicted to bf16 with a
                              ones column per head for the softmax sum)
  g     = q_x @ Wg^T + bg    (bg added via a rank-1 K=1 matmul; sigmoid done
                              as 0.5*(1+tanh(x/2)) with the 0.5 folded into Wo)
  sT_h  = kT_h^T @ qT_h      (scores transposed: [k, q], per head)
  expS  = exp(sT + mask)     (ACT, mask is the per-partition bias operand;
                              no max-subtraction -- |logits| <= ~70 so
                              fp32->bf16 exp cannot overflow)
  A     = expS * exp(pairT)  (exp(bias_pair) precomputed on host, bf16)
  o,Z   = A_h^T @ [v_h | 1]  (AV matmul in natural layout, N=33 per head;
                              col 32 accumulates Z = sum_k A)
  og    = (tanh(g/2)+1) * (o * (1/Z))
  ogT   = PE transpose(og)
  out   = ogT^T @ (0.5*Wo)^T + bo
"""

import os
import numpy as np
import ml_dtypes

def _mmdt():
    return (ml_dtypes.bfloat16 if os.environ.get('KDTYPE', 'fp16') == 'bf16'
            else np.float16)

B, S, Q, C = 1, 128, 256, 256
H, DH = 8, 32
TOT = H * DH
N_CORES = 8
S_LOC = S // N_CORES  # 16

_CACHE = {}


def _build_program(s_loc):
    import concourse.bacc as bacc
    import concourse.mybir as mybir
    from concourse import tile

    dt = mybir.dt
    f32, bf16 = dt.float32, dt.bfloat16
    f16 = bf16 if os.environ.get('KDTYPE', 'fp16') == 'bf16' else dt.float16
    AF = mybir.ActivationFunctionType
    ALU = mybir.AluOpType

    nc = bacc.Bacc("TRN2", target_bir_lowering=False, debug=False,
                   num_devices=N_CORES)

    x_d = nc.dram_tensor("x", [s_loc, 2 * C, Q], f16, kind="ExternalInput").ap()
    mask_d = nc.dram_tensor("maskt", [128, 2 * s_loc], f32, kind="ExternalInput").ap()
    expb_d = nc.dram_tensor("expb", [128, 2 * H * Q], bf16, kind="ExternalInput").ap()
    wq_d = nc.dram_tensor("wq", [128, 512], f16, kind="ExternalInput").ap()
    wk_d = nc.dram_tensor("wk", [128, 512], f16, kind="ExternalInput").ap()
    wv_d = nc.dram_tensor("wv", [128, 512], f16, kind="ExternalInput").ap()
    wg_d = nc.dram_tensor("wg", [128, 512], f16, kind="ExternalInput").ap()
    wo_d = nc.dram_tensor("wo", [128, 512], f16, kind="ExternalInput").ap()
    bg_d = nc.dram_tensor("bg", [1, 256], f16, kind="ExternalInput").ap()
    bo_d = nc.dram_tensor("bo", [128, 256], f32, kind="ExternalInput").ap()
    id_d = nc.dram_tensor("ident", [128, 128], f16, kind="ExternalInput").ap()
    ones_d = nc.dram_tensor("ones", [1, 128], f16, kind="ExternalInput").ap()
    out_d = nc.dram_tensor("out", [s_loc, Q, C], f32, kind="ExternalOutput").ap()

    def r(ap):
        return ap

    with tile.TileContext(nc) as tc:
        with (
            tc.tile_pool(name="const", bufs=1) as cp,
            tc.tile_pool(name="work", bufs=2) as wp,
            tc.tile_pool(name="ps_small", bufs=2, space="PSUM") as pss,
            tc.tile_pool(name="ps_sc", bufs=2, space="PSUM") as psc,
            tc.tile_pool(name="ps_o", bufs=2, space="PSUM") as pso,
        ):
            # ---- resident constants ----
            wq_t = cp.tile([128, 512], f16, tag="wq")
            wk_t = cp.tile([128, 512], f16, tag="wk")
            wv_t = cp.tile([128, 512], f16, tag="wv")
            wg_t = cp.tile([128, 512], f16, tag="wg")
            wo_t = cp.tile([128, 512], f16, tag="wo")
            expb_t = cp.tile([128, 2 * H * Q], bf16, tag="expb")
            mask_t = cp.tile([128, 2 * s_loc], f32, tag="mask")
            bg_t = cp.tile([1, 256], f16, tag="bg")
            bo_t = cp.tile([128, 256], f32, tag="bo")
            id_t = cp.tile([128, 128], f16, tag="ident")
            ones_t = cp.tile([1, 128], f16, tag="ones")

            nc.sync.dma_start(wq_t[:, :], wq_d[:, :])
            nc.sync.dma_start(wk_t[:, :], wk_d[:, :])
            nc.sync.dma_start(wv_t[:, :], wv_d[:, :])
            nc.sync.dma_start(wg_t[:, :], wg_d[:, :])
            nc.sync.dma_start(wo_t[:, :], wo_d[:, :])
            nc.sync.dma_start(expb_t[:, :], expb_d[:, :])
            nc.sync.dma_start(mask_t[:, :], mask_d[:, :])
            nc.sync.dma_start(bg_t[:, :], bg_d[:, :])
            nc.sync.dma_start(bo_t[:, :], bo_d[:, :])
            nc.sync.dma_start(id_t[:, :], id_d[:, :])
            nc.sync.dma_start(ones_t[:, :], ones_d[:, :])

            for s in range(s_loc):
                # ---- load x^T shards (xq | xkv in one tensor) ----
                xx = wp.tile([128, 1024], f16, tag="xx")
                nc.sync.dma_start(
                    xx[:, :].rearrange("p (cc q) -> p cc q", cc=4),
                    x_d[s].rearrange("(cc p) q -> p cc q", p=128))
                xq = xx[:, 0:512]
                xkv = xx[:, 512:1024]

                # ---- projections (fp32r) ----
                # qT[t, q] += WqT[c, t]^T @ xqT[c, q]
                qt_ps = pss.tile([128, 512], f32, tag="pss")
                for tcc in range(2):
                    for cc in range(2):
                        nc.tensor.matmul(
                            qt_ps[:, tcc * 256:(tcc + 1) * 256],
                            r(wq_t[:, cc * 256 + tcc * 128: cc * 256 + tcc * 128 + 128]),
                            r(xq[:, cc * 256:(cc + 1) * 256]),
                            start=(cc == 0), stop=(cc == 1))
                qt = wp.tile([128, 512], f16, tag="qt")
                ev_q = nc.vector.tensor_copy(qt[:, :], qt_ps[:, :])

                kt_ps = pss.tile([128, 512], f32, tag="pss")
                for tcc in range(2):
                    for cc in range(2):
                        nc.tensor.matmul(
                            kt_ps[:, tcc * 256:(tcc + 1) * 256],
                            r(wk_t[:, cc * 256 + tcc * 128: cc * 256 + tcc * 128 + 128]),
                            r(xkv[:, cc * 256:(cc + 1) * 256]),
                            start=(cc == 0), stop=(cc == 1))
                kt = wp.tile([128, 512], f16, tag="kt")
                ev_k = nc.vector.tensor_copy(kt[:, :], kt_ps[:, :])

                # v natural [k, t]
                v_ps = pss.tile([128, 512], f32, tag="pss")
                for kc in range(2):
                    for cc in range(2):
                        nc.tensor.matmul(
                            v_ps[:, kc * 256:(kc + 1) * 256],
                            r(xkv[:, cc * 256 + kc * 128: cc * 256 + kc * 128 + 128]),
                            r(wv_t[:, cc * 256:(cc + 1) * 256]),
                            start=(cc == 0), stop=(cc == 1))
                # v_aug bf16 [k, (kc, h, 33)]; col 32 of each head = 1.0
                v_sb = wp.tile([128, 528], bf16, tag="v")
                v4 = v_sb.rearrange("p (kc h e) -> p kc h e", kc=2, h=8)
                nc.gpsimd.memset(v4[:, :, :, 32], 1.0)
                for kc in range(2):
                    nc.vector.tensor_copy(
                        v4[:, kc, :, 0:32],
                        v_ps[:, kc * 256:(kc + 1) * 256].rearrange(
                            "p (h d) -> p h d", h=8))

                # g natural [q, t] with bg via rank-1 matmul
                g_ps = pss.tile([128, 512], f32, tag="pss")
                for qc in range(2):
                    for cc in range(2):
                        nc.tensor.matmul(
                            g_ps[:, qc * 256:(qc + 1) * 256],
                            r(xq[:, cc * 256 + qc * 128: cc * 256 + qc * 128 + 128]),
                            r(wg_t[:, cc * 256:(cc + 1) * 256]),
                            start=(cc == 0), stop=False)
                    nc.tensor.matmul(
                        g_ps[:, qc * 256:(qc + 1) * 256],
                        r(ones_t[:, :]), r(bg_t[:, :]),
                        start=False, stop=True)
                # gs = tanh(g/2); sigmoid = 0.5*(gs+1), 0.5 folded into Wo
                gs = wp.tile([128, 512], f32, tag="gs")
                nc.scalar.activation(gs[:, :], g_ps[:, :], AF.Tanh, scale=0.5)

                # ---- attention ----
                # Mixing PE tile positions crashes this runtime, so every
                # matmul must sit at partition base 0: DMA-remap qt/kt from
                # [(hh,d), (tc,q)] to head-flat [d, (tc,hh,q)].
                qt2 = wp.tile([32, 2048], f16, tag="qt2")
                kt2 = wp.tile([32, 2048], f16, tag="kt2")
                # A DMA source AP cannot stride across partitions in a
                # non-leading dim, so remap per head-quarter (both t-chunks
                # in one 3D-AP DMA). Triggers go on otherwise-idle engines
                # to keep the sync queue off the critical path.
                qeng = (nc.gpsimd, nc.scalar, nc.gpsimd, nc.sync)
                keng = (nc.scalar, nc.sync, nc.scalar, nc.sync)
                for hh_ in range(4):
                    srcp = slice(hh_ * 32, hh_ * 32 + 32)
                    dstc = slice(hh_ * 512, hh_ * 512 + 512)
                    qeng[hh_].dma_start(qt2[:, dstc], qt[srcp, :])
                    keng[hh_].dma_start(kt2[:, dstc], kt[srcp, :])

                expS = wp.tile([128, 4096], bf16, tag="expS")
                for hg in range(2):
                    for kc in range(2):
                        sc_ps = psc.tile([128, 1024], f32, tag="sc")
                        for hh in range(4):
                            h = hg * 4 + hh
                            tch, hhh = h // 4, h % 4
                            base = hhh * 512 + tch * 256
                            nc.tensor.matmul(
                                sc_ps[:, hh * 256:(hh + 1) * 256],
                                kt2[:, base + kc * 128: base + kc * 128 + 128],
                                qt2[:, base: base + 256],
                                start=True, stop=True)
                        # exp(s + mask_k) -> bf16
                        nc.scalar.activation(
                            expS[:, kc * 2048 + hg * 1024:
                                 kc * 2048 + (hg + 1) * 1024],
                            sc_ps[:, :], AF.Exp,
                            bias=mask_t[:, kc * s_loc + s: kc * s_loc + s + 1])
                # A = expS * exp(pair bias)
                A = wp.tile([128, 4096], bf16, tag="A")
                for kc in range(2):
                    nc.vector.tensor_mul(
                        A[:, kc * 2048:(kc + 1) * 2048],
                        expS[:, kc * 2048:(kc + 1) * 2048],
                        expb_t[:, kc * 2048:(kc + 1) * 2048])

                # AV: o[q, (h,33)] += A_h^T @ [v_h | 1]
                o_ps = {}
                for qc in range(2):
                    o_ps[qc] = pso.tile([128, 264], f32, tag="o",
                                        name=f"o{qc}")
                for h in range(H):
                    for qc in range(2):
                        for kc in range(2):
                            nc.tensor.matmul(
                                o_ps[qc][:, h * 33: h * 33 + 33],
                                A[:, kc * 2048 + h * 256 + qc * 128:
                                   kc * 2048 + h * 256 + qc * 128 + 128],
                                v_sb[:, kc * 264 + h * 33: kc * 264 + h * 33 + 33],
                                start=(kc == 0), stop=(kc == 1))

                # normalize + gate: og = (gs+1) * (o * (1/Z))
                rz = wp.tile([128, 16], f32, tag="rz")
                t1 = wp.tile([128, 512], f32, tag="t1")
                for qc in range(2):
                    o3 = o_ps[qc].rearrange("p (h e) -> p h e", h=8)
                    nc.vector.reciprocal(
                        rz[:, qc * 8:(qc + 1) * 8], o3[:, :, 32])
                    nc.vector.tensor_mul(
                        t1[:, qc * 256:(qc + 1) * 256].rearrange(
                            "p (h d) -> p h d", h=8),
                        o3[:, :, 0:32],
                        rz[:, qc * 8:(qc + 1) * 8].unsqueeze(2).broadcast_to(
                            (128, 8, 32)))
                og = wp.tile([128, 512], f16, tag="og")
                nc.vector.scalar_tensor_tensor(
                    og[:, :], gs[:, :], 1.0, t1[:, :],
                    op0=ALU.add, op1=ALU.mult)

                # transpose og -> ogT via PE
                tr_ps = pss.tile([128, 512], f16, tag="pss")
                for tcc in range(2):
                    for qc in range(2):
                        nc.tensor.transpose(
                            tr_ps[:, tcc * 256 + qc * 128: tcc * 256 + qc * 128 + 128],
                            og[:, qc * 256 + tcc * 128: qc * 256 + tcc * 128 + 128],
                            id_t[:, :])
                ogt = wp.tile([128, 512], f16, tag="ogt")
                nc.vector.tensor_copy(ogt[:, :], tr_ps[:, :])

                # final projection + bo
                f_ps = pss.tile([128, 512], f32, tag="pss")
                for qc in range(2):
                    for tcc in range(2):
                        nc.tensor.matmul(
                            f_ps[:, qc * 256:(qc + 1) * 256],
                            r(ogt[:, tcc * 256 + qc * 128: tcc * 256 + qc * 128 + 128]),
                            r(wo_t[:, tcc * 256:(tcc + 1) * 256]),
                            start=(tcc == 0), stop=(tcc == 1))
                out_sb = wp.tile([128, 512], f32, tag="out")
                nc.vector.tensor_tensor(
                    out_sb[:, :].rearrange("p (qc c) -> p qc c", qc=2),
                    f_ps[:, :].rearrange("p (qc c) -> p qc c", qc=2),
                    bo_t[:, :].unsqueeze(1).broadcast_to((128, 2, 256)),
                    op=ALU.add)
                nc.sync.dma_start(
                    out_d[s].rearrange("(qc p) c -> p qc c", p=128),
                    out_sb[:, :].rearrange("p (qc c) -> p qc c", qc=2))

    nc.compile()
    return nc


def get_program(s_loc=S_LOC):
    key = (s_loc, os.environ.get('KDTYPE', 'bf16'))
    if key not in _CACHE:
        _CACHE[key] = _build_program(s_loc)
    return _CACHE[key]


def prep_inputs(q_x, kv_x, bias_mask, bias_pair, Wq, Wk, Wv, Wg, bg, Wo, bo,
                s_loc=S_LOC, n_cores=N_CORES):
    """Host-side layout prep. Returns per-core in_maps."""
    bf16 = ml_dtypes.bfloat16

    def wprep(wt):  # (C_in, T_out) -> [p, (cc, t)]
        return np.ascontiguousarray(
            wt.reshape(2, 128, 256).transpose(1, 0, 2).reshape(128, 512)
        ).astype(_mmdt())

    wq_h = wprep(np.asarray(Wq).T)     # lhsT[c, t] = Wq[t, c]
    wk_h = wprep(np.asarray(Wk).T)
    wv_h = wprep(np.asarray(Wv).T)     # rhs[c, t]
    wg_h = wprep(np.asarray(Wg).T)
    wo_h = wprep(np.asarray(Wo).T * 0.5)  # rhs[t, c] = Wo[c, t]; 0.5 = sigmoid fold
    bg_h = np.asarray(bg, _mmdt()).reshape(1, 256)
    bo_h = np.ascontiguousarray(np.broadcast_to(
        np.asarray(bo, np.float32), (128, 256)))
    id_h = np.eye(128, dtype=_mmdt())

    eb = np.exp(np.asarray(bias_pair[0, 0], np.float64)).astype(np.float32)
    ebT = eb.transpose(0, 2, 1)  # (H, K, Q)
    expb_h = np.ascontiguousarray(
        ebT.reshape(H, 2, 128, Q).transpose(2, 1, 0, 3).reshape(128, 2 * H * Q)
    ).astype(bf16)

    x_all = np.concatenate([
        np.asarray(q_x[0], _mmdt()).transpose(0, 2, 1),
        np.asarray(kv_x[0], _mmdt()).transpose(0, 2, 1)], axis=1)
    x_all = np.ascontiguousarray(x_all)   # (S, 2C, Q): xq | xkv
    mask_all = np.asarray(bias_mask[0, :, 0, 0, :], np.float32)  # (S, K)

    in_maps = []
    for core in range(n_cores):
        lo = core * s_loc
        m = mask_all[lo:lo + s_loc]  # (s_loc, K)
        mask_h = np.ascontiguousarray(
            m.T.reshape(2, 128, s_loc).transpose(1, 0, 2).reshape(128, 2 * s_loc))
        in_maps.append({
            "x": x_all[lo:lo + s_loc],
            "maskt": mask_h,
            "expb": expb_h,
            "wq": wq_h, "wk": wk_h, "wv": wv_h, "wg": wg_h, "wo": wo_h,
            "bg": bg_h, "bo": bo_h, "ident": id_h,
            "ones": np.ones((1, 128), _mmdt()),
        })
    return in_maps


def kernel(q_x, kv_x, bias_mask, bias_pair, Wq, Wk, Wv, Wg, bg, Wo, bo):
    from concourse import bass_utils

    nc = get_program()
    in_maps = prep_inputs(q_x, kv_x, bias_mask, bias_pair,
                          Wq, Wk, Wv, Wg, bg, Wo, bo)
    res = bass_utils.run_bass_kernel_spmd(
        nc, in_maps, core_ids=list(range(N_CORES)))
    out = np.concatenate([res.results[i]["out"] for i in range(N_CORES)], axis=0)
    return out.reshape(B, S, Q, C).astype(np.float32)



# revision 3
# speedup vs baseline: 1.2123x; 1.2123x over previous
"""Trainium2 Bass kernel for the sparse_attention nn.Module problem.

Strategy: data-parallel over the MSA-row dim S (S=128 -> 16 rows per core,
8 cores). All projection weights + pair bias replicated; activations and
mask sharded with S. No collectives.

Per-core dataflow (scheme C -- fully transposed attention, tile_position
packed matmuls, mask folded into v / Z so exp needs no bias):
  qT/kT/gT = W @ x^T          (PSUM f32, DVE/ACT evict to fp16; partition
                               layout [(hh,d), (tc, q)] feeds packed scores
                               directly -- no SBUF remap DMAs)
  v'       = (kv_x @ Wv^T) * exp(mask)[k]   (mask folded into v rows)
  sT_h     = kT_h^T @ qT_h    (4-way ROW-packed tile_position=(32hh,0);
                               each concurrent MM drains to its own PSUM
                               bank: out col = hh*512 + kc*256)
  expS     = exp(sT)          (no bias -> one ACT op spans all 4 banks,
                               2 ops/row of [128,2048])
  A        = expS * exp(pair) (DVE bf16; exp(pair) precomputed on host)
  oT_h     = v'_h^T @ A_h     (4-way COL-packed tile_position=(0,32hh),
                               M=32, out partitions 32hh..+32)
  Zbc_h    = em^T @ A_h       (same col-packing with lhsT = exp(mask)
                               replicated 32 cols -> Z_h[q] lands on the
                               SAME partitions as oT_h: lane-aligned)
  og       = (tanh(gT/2)+1) * oT * recip(Zbc)   (0.5 folded into Wo; bg
                               added to gT via rank-1 K=1 matmuls)
  out      = og^T @ (0.5*Wo)^T + bo             (bo via rank-1 matmul)
"""

import os
import numpy as np
import ml_dtypes

B, S, Q, C = 1, 128, 256, 256
H, DH = 8, 32
TOT = H * DH
N_CORES = 8
S_LOC = S // N_CORES  # 16

_CACHE = {}


def _build_program(s_loc):
    import concourse.bacc as bacc
    import concourse.mybir as mybir
    from concourse import tile

    dt = mybir.dt
    f32, bf16, f16 = dt.float32, dt.bfloat16, dt.float16
    AF = mybir.ActivationFunctionType
    ALU = mybir.AluOpType

    nc = bacc.Bacc("TRN2", target_bir_lowering=False, debug=False,
                   num_devices=N_CORES)

    x_d = nc.dram_tensor("x", [s_loc, 2 * C, Q], f16, kind="ExternalInput").ap()
    wq_d = nc.dram_tensor("wq", [128, 512], f16, kind="ExternalInput").ap()
    wk_d = nc.dram_tensor("wk", [128, 512], f16, kind="ExternalInput").ap()
    wv_d = nc.dram_tensor("wv", [128, 512], f16, kind="ExternalInput").ap()
    wg_d = nc.dram_tensor("wg", [128, 512], f16, kind="ExternalInput").ap()
    wo_d = nc.dram_tensor("wo", [128, 512], f16, kind="ExternalInput").ap()
    expb_d = nc.dram_tensor("expb", [128, 4096], bf16, kind="ExternalInput").ap()
    em_d = nc.dram_tensor("em", [128, s_loc * 64], bf16, kind="ExternalInput").ap()
    sml_d = nc.dram_tensor("small", [1, 768], f16, kind="ExternalInput").ap()
    out_d = nc.dram_tensor("out", [s_loc, Q, C], f32, kind="ExternalOutput").ap()

    with tile.TileContext(nc) as tc:
        with (
            tc.tile_pool(name="const", bufs=1) as cp,
            tc.tile_pool(name="work", bufs=2) as wp,
            tc.tile_pool(name="ps_small", bufs=4, space="PSUM") as pss,
            tc.tile_pool(name="ps_sc", bufs=1, space="PSUM") as psc,
        ):
            # ---- resident constants ----
            wq_t = cp.tile([128, 512], f16, tag="wq")
            wk_t = cp.tile([128, 512], f16, tag="wk")
            wv_t = cp.tile([128, 512], f16, tag="wv")
            wg_t = cp.tile([128, 512], f16, tag="wg")
            wo_t = cp.tile([128, 512], f16, tag="wo")
            expb_t = cp.tile([128, 4096], bf16, tag="expb")
            em_t = cp.tile([128, s_loc * 64], bf16, tag="em")
            sml_t = cp.tile([1, 768], f16, tag="small")

            nc.sync.dma_start(wq_t[:, :], wq_d[:, :])
            nc.sync.dma_start(wk_t[:, :], wk_d[:, :])
            nc.sync.dma_start(wv_t[:, :], wv_d[:, :])
            nc.sync.dma_start(wg_t[:, :], wg_d[:, :])
            nc.sync.dma_start(wo_t[:, :], wo_d[:, :])
            nc.sync.dma_start(expb_t[:, :], expb_d[:, :])
            nc.sync.dma_start(em_t[:, :], em_d[:, :])
            nc.sync.dma_start(sml_t[:, :], sml_d[:, :])
            bg_l = sml_t[:, 0:256]     # lhsT cols = t, for gT bias
            ones_r = sml_t[:, 256:512]  # rhs for gT bias / lhsT[0:128] for bo
            bo_r = sml_t[:, 512:768]   # rhs cols = c, for bo add

            for s in range(s_loc):
                # ---- load x^T shards (xq | xkv in one tensor) ----
                xx = wp.tile([128, 1024], f16, tag="xx")
                nc.sync.dma_start(
                    xx[:, :].rearrange("p (cc q) -> p cc q", cc=4),
                    x_d[s].rearrange("(cc p) q -> p cc q", p=128))
                xq = xx[:, 0:512]
                xkv = xx[:, 512:1024]

                # ---- projections ----
                # qT/kT/gT[t, q] += W^T[c, t]^T @ x^T[c, q]
                def proj_T(w_t, dst_tag, dst_dt, src):
                    ps = pss.tile([128, 512], f32, tag="pss")
                    for tcc in range(2):
                        for cc in range(2):
                            nc.tensor.matmul(
                                ps[:, tcc * 256:(tcc + 1) * 256],
                                w_t[:, cc * 256 + tcc * 128:
                                    cc * 256 + tcc * 128 + 128],
                                src[:, cc * 256:(cc + 1) * 256],
                                start=(cc == 0), stop=(cc == 1))
                    return ps

                qt_ps = proj_T(wq_t, "qt", f16, xq)
                qt = wp.tile([128, 512], f16, tag="qt")
                nc.vector.tensor_copy(qt[:, :], qt_ps[:, :])

                kt_ps = proj_T(wk_t, "kt", f16, xkv)
                kt = wp.tile([128, 512], f16, tag="kt")
                nc.vector.tensor_copy(kt[:, :], kt_ps[:, :])

                # gT with bg added via rank-1 K=1 matmul (lhsT = bg cols)
                g_ps = pss.tile([128, 512], f32, tag="pss")
                for tcc in range(2):
                    for cc in range(2):
                        nc.tensor.matmul(
                            g_ps[:, tcc * 256:(tcc + 1) * 256],
                            wg_t[:, cc * 256 + tcc * 128:
                                 cc * 256 + tcc * 128 + 128],
                            xq[:, cc * 256:(cc + 1) * 256],
                            start=(cc == 0), stop=False)
                    nc.tensor.matmul(
                        g_ps[:, tcc * 256:(tcc + 1) * 256],
                        bg_l[:, tcc * 128:tcc * 128 + 128],
                        ones_r[:, :],
                        start=False, stop=True)
                # gs = tanh((g+bg)/2); sigmoid = 0.5*(gs+1), 0.5 in Wo
                gs = wp.tile([128, 512], f16, tag="gs")
                nc.scalar.activation(gs[:, :], g_ps[:, :], AF.Tanh, scale=0.5)

                # v natural [k, (kc, t)] then v' = v * exp(mask)[k]
                v_ps = pss.tile([128, 512], f32, tag="pss")
                for kc in range(2):
                    for cc in range(2):
                        nc.tensor.matmul(
                            v_ps[:, kc * 256:(kc + 1) * 256],
                            xkv[:, cc * 256 + kc * 128:
                                cc * 256 + kc * 128 + 128],
                            wv_t[:, cc * 256:(cc + 1) * 256],
                            start=(cc == 0), stop=(cc == 1))
                vs = wp.tile([128, 512], bf16, tag="vs")
                for kc in range(2):
                    nc.vector.scalar_tensor_tensor(
                        vs[:, kc * 256:(kc + 1) * 256],
                        v_ps[:, kc * 256:(kc + 1) * 256], 1.0,
                        em_t[:, s * 64 + kc * 32:s * 64 + kc * 32 + 1
                             ].broadcast_to((128, 256)),
                        op0=ALU.mult, op1=ALU.mult)

                # ---- attention (per head-group hg = t-chunk) ----
                og = wp.tile([128, 512], f16, tag="og")
                for hg in range(2):
                    # scores: 4-way row-packed; out col hh*512+kc*256
                    # puts each concurrent MM in its own PSUM bank
                    sc_ps = psc.tile([128, 2048], f32, tag="sc")
                    for kc in range(2):
                        for hh in range(4):
                            nc.tensor.matmul(
                                sc_ps[:, hh * 512 + kc * 256:
                                      hh * 512 + kc * 256 + 256],
                                kt[32 * hh:32 * hh + 32,
                                   hg * 256 + kc * 128:
                                   hg * 256 + kc * 128 + 128],
                                qt[32 * hh:32 * hh + 32,
                                   hg * 256:hg * 256 + 256],
                                start=True, stop=True,
                                tile_position=(32 * hh, 0))
                    # exp over all 4 banks in one ACT op (no bias needed)
                    expS = wp.tile([128, 2048], bf16, tag="expS")
                    nc.scalar.activation(expS[:, :], sc_ps[:, :], AF.Exp)
                    # A = expS * exp(pair bias)
                    A = wp.tile([128, 2048], bf16, tag="A")
                    nc.vector.tensor_tensor(
                        A[:, :], expS[:, :],
                        expb_t[:, hg * 2048:(hg + 1) * 2048], op=ALU.mult)

                    # AV + Z: col-packed M=32, out partitions 32hh..+32.
                    # ovz[:, 0:256] = oT, ovz[:, 256:512] = Z replicated
                    # (same partitions as oT -> lane-aligned divide).
                    ovz = pss.tile([128, 512], f32, tag="pss")
                    for hh in range(4):
                        for kc in range(2):
                            nc.tensor.matmul(
                                ovz[32 * hh:32 * hh + 32, 0:256],
                                vs[:, kc * 256 + (hg * 4 + hh) * 32:
                                   kc * 256 + (hg * 4 + hh) * 32 + 32],
                                A[:, hh * 512 + kc * 256:
                                  hh * 512 + kc * 256 + 256],
                                start=(kc == 0), stop=(kc == 1),
                                tile_position=(0, 32 * hh))
                    for hh in range(4):
                        for kc in range(2):
                            nc.tensor.matmul(
                                ovz[32 * hh:32 * hh + 32, 256:512],
                                em_t[:, s * 64 + kc * 32:
                                     s * 64 + kc * 32 + 32],
                                A[:, hh * 512 + kc * 256:
                                  hh * 512 + kc * 256 + 256],
                                start=(kc == 0), stop=(kc == 1),
                                tile_position=(0, 32 * hh))

                    # og = (gs+1) * oT * recip(Z)
                    rz = wp.tile([128, 256], f32, tag="rz")
                    nc.vector.reciprocal_approx_fast(rz[:, :], ovz[:, 256:512])
                    t_sb = wp.tile([128, 256], f16, tag="tsb")
                    nc.vector.tensor_tensor(
                        t_sb[:, :], ovz[:, 0:256], rz[:, :], op=ALU.mult)
                    nc.vector.scalar_tensor_tensor(
                        og[:, hg * 256:(hg + 1) * 256],
                        gs[:, hg * 256:(hg + 1) * 256], 1.0, t_sb[:, :],
                        op0=ALU.add, op1=ALU.mult)

                # ---- final projection + bo (rank-1 matmul) ----
                f_ps = pss.tile([128, 512], f32, tag="pss")
                for qc in range(2):
                    for tcc in range(2):
                        nc.tensor.matmul(
                            f_ps[:, qc * 256:(qc + 1) * 256],
                            og[:, tcc * 256 + qc * 128:
                               tcc * 256 + qc * 128 + 128],
                            wo_t[:, tcc * 256:(tcc + 1) * 256],
                            start=(tcc == 0), stop=False)
                    nc.tensor.matmul(
                        f_ps[:, qc * 256:(qc + 1) * 256],
                        ones_r[:, 0:128], bo_r[:, :],
                        start=False, stop=True)
                out_sb = wp.tile([128, 512], f32, tag="out")
                nc.vector.tensor_copy(out_sb[:, :], f_ps[:, :])
                nc.gpsimd.dma_start(
                    out_d[s].rearrange("(qc p) c -> p qc c", p=128),
                    out_sb[:, :].rearrange("p (qc c) -> p qc c", qc=2))

    nc.compile()
    return nc


def get_program(s_loc=S_LOC):
    key = s_loc
    if key not in _CACHE:
        _CACHE[key] = _build_program(s_loc)
    return _CACHE[key]


def prep_inputs(q_x, kv_x, bias_mask, bias_pair, Wq, Wk, Wv, Wg, bg, Wo, bo,
                s_loc=S_LOC, n_cores=N_CORES):
    """Host-side layout prep. Returns per-core in_maps."""
    bf16 = ml_dtypes.bfloat16
    f16 = np.float16

    def wprep(wt):  # (in_dim, out_dim) -> [p, (cc, out)]
        return np.ascontiguousarray(
            wt.reshape(2, 128, 256).transpose(1, 0, 2).reshape(128, 512)
        ).astype(f16)

    wq_h = wprep(np.asarray(Wq).T)     # lhsT[c, t] = Wq[t, c]
    wk_h = wprep(np.asarray(Wk).T)
    wg_h = wprep(np.asarray(Wg).T)
    wv_h = wprep(np.asarray(Wv).T)     # rhs[c, t]
    wo_h = wprep(np.asarray(Wo).T * 0.5)  # rhs[t, c]; 0.5 = sigmoid fold

    sml = np.zeros((1, 768), f16)
    sml[0, 0:256] = np.asarray(bg, f16)
    sml[0, 256:512] = 1.0
    sml[0, 512:768] = np.asarray(bo, f16)

    # exp(pair)^T as [128, (hg, hh, kc, q)]
    eb = np.exp(np.asarray(bias_pair[0, 0], np.float64))  # (H, Q, K)
    ebT = eb.transpose(0, 2, 1)  # (H, K, Q)
    expb_h = np.ascontiguousarray(
        ebT.reshape(2, 4, 2, 128, Q).transpose(3, 0, 1, 2, 4).reshape(128, 4096)
    ).astype(bf16)

    x_all = np.concatenate([
        np.asarray(q_x[0], f16).transpose(0, 2, 1),
        np.asarray(kv_x[0], f16).transpose(0, 2, 1)], axis=1)
    x_all = np.ascontiguousarray(x_all)   # (S, 2C, Q): xq | xkv
    # exp(mask) replicated 32x: [128, (s, kc, 32)]
    em_all = np.exp(np.asarray(bias_mask[0, :, 0, 0, :], np.float64))  # (S, K)

    in_maps = []
    for core in range(n_cores):
        lo = core * s_loc
        em = em_all[lo:lo + s_loc].reshape(s_loc, 2, 128)  # (s, kc, p)
        em_h = np.ascontiguousarray(np.broadcast_to(
            em.transpose(2, 0, 1)[:, :, :, None], (128, s_loc, 2, 32)
        ).reshape(128, s_loc * 64)).astype(bf16)
        in_maps.append({
            "x": x_all[lo:lo + s_loc],
            "em": em_h, "expb": expb_h,
            "wq": wq_h, "wk": wk_h, "wv": wv_h, "wg": wg_h, "wo": wo_h,
            "small": sml,
        })
    return in_maps


def kernel(q_x, kv_x, bias_mask, bias_pair, Wq, Wk, Wv, Wg, bg, Wo, bo):
    from concourse import bass_utils

    nc = get_program()
    in_maps = prep_inputs(q_x, kv_x, bias_mask, bias_pair,
                          Wq, Wk, Wv, Wg, bg, Wo, bo)
    res = bass_utils.run_bass_kernel_spmd(
        nc, in_maps, core_ids=list(range(N_CORES)))
    out = np.concatenate([res.results[i]["out"] for i in range(N_CORES)], axis=0)
    return out.reshape(B, S, Q, C).astype(np.float32)


# revision 6
# speedup vs baseline: 1.3201x; 1.0889x over previous
"""Trainium2 Bass kernel for the sparse_attention nn.Module problem.

Strategy: data-parallel over the MSA-row dim S (S=128 -> 16 rows per core,
8 cores). All projection weights + pair bias replicated; activations and
mask sharded with S. No collectives.

Per-core dataflow (scheme C2 -- fully transposed attention, tile_position
packed matmuls, mask folded into v / Z so exp needs no bias, row-PAIRED
projections so the shared weights stream N=512, software-pipelined
emission: projections of pair p interleave with attention of pair p-1):
  qT/kT/gT = W @ [x_s0^T | x_s1^T]   (paired N=512 matmuls, PSUM f32,
                                      DVE evict fp16 / ACT tanh for gate;
                                      bg folded into the ACT bias)
  v'_s     = (kv_x @ Wv^T) * exp(mask)[k]    (mask folded into v rows)
  sT_h     = kT_h^T @ qT_h    (4-way ROW-packed tile_position=(32hh,0);
                               concurrent MMs drain into 4 distinct PSUM
                               banks across two 2-bank tiles scA/scB)
  expS     = exp(sT)          (no bias -> [128,1024] ACT ops)
  A        = expS * exp(pair) (DVE bf16; one chunk per row on GpSimd)
  oT_h     = v'_h^T @ A_h     (4-way COL-packed tile_position=(0,32hh))
  Zbc_h    = em^T @ A_h       (same col-packing, lhsT = exp(mask) x32
                               -> Z_h[q] lane-aligned with oT_h)
  og       = (tanh((gT+bg)/2)+1) * oT / Zbc   (0.5 folded into Wo)
  out      = og^T @ (0.5*Wo)^T + bo           (bo added in the eviction)
"""

import os
import numpy as np
import ml_dtypes

B, S, Q, C = 1, 128, 256, 256
H, DH = 8, 32
TOT = H * DH
N_CORES = 8
S_LOC = S // N_CORES  # 16

_CACHE = {}


def _build_program(s_loc):
    import concourse.bacc as bacc
    import concourse.mybir as mybir
    from concourse import tile

    dt = mybir.dt
    f32, bf16, f16 = dt.float32, dt.bfloat16, dt.float16
    AF = mybir.ActivationFunctionType
    ALU = mybir.AluOpType
    use_div = os.environ.get("KDIV", "recip") == "div"
    gp_mul = int(os.environ.get("KGP", "1"))  # A-mul chunks on GpSimd /row

    npair = s_loc // 2

    nc = bacc.Bacc("TRN2", target_bir_lowering=False, debug=False,
                   num_devices=N_CORES)

    x_d = nc.dram_tensor("x", [s_loc, 2 * C, Q], f16, kind="ExternalInput").ap()
    wq_d = nc.dram_tensor("wq", [128, 512], f16, kind="ExternalInput").ap()
    wk_d = nc.dram_tensor("wk", [128, 512], f16, kind="ExternalInput").ap()
    wv_d = nc.dram_tensor("wv", [128, 512], f16, kind="ExternalInput").ap()
    wg_d = nc.dram_tensor("wg", [128, 512], f16, kind="ExternalInput").ap()
    wo_d = nc.dram_tensor("wo", [128, 512], f16, kind="ExternalInput").ap()
    expb_d = nc.dram_tensor("expb", [128, 4096], bf16, kind="ExternalInput").ap()
    em_d = nc.dram_tensor("em", [128, s_loc * 64], bf16, kind="ExternalInput").ap()
    bgc_d = nc.dram_tensor("bgc", [128, 2], f32, kind="ExternalInput").ap()
    bo_d = nc.dram_tensor("bo", [128, 256], f32, kind="ExternalInput").ap()
    out_d = nc.dram_tensor("out", [s_loc, Q, C], f32, kind="ExternalOutput").ap()

    with tile.TileContext(nc) as tc:
        with (
            tc.tile_pool(name="const", bufs=1) as cp,
            tc.tile_pool(name="work", bufs=2) as wp,
            tc.tile_pool(name="work4", bufs=4) as wp4,
            tc.tile_pool(name="pp", bufs=2, space="PSUM") as pp,
            tc.tile_pool(name="sca", bufs=1, space="PSUM") as pscA,
            tc.tile_pool(name="scb", bufs=1, space="PSUM") as pscB,
        ):
            # ---- resident constants ----
            wq_t = cp.tile([128, 512], f16, tag="wq")
            wk_t = cp.tile([128, 512], f16, tag="wk")
            wv_t = cp.tile([128, 512], f16, tag="wv")
            wg_t = cp.tile([128, 512], f16, tag="wg")
            wo_t = cp.tile([128, 512], f16, tag="wo")
            expb_t = cp.tile([128, 4096], bf16, tag="expb")
            em_t = cp.tile([128, s_loc * 64], bf16, tag="em")
            bgc_t = cp.tile([128, 2], f32, tag="bgc")
            bo_t = cp.tile([128, 256], f32, tag="bo")

            nc.sync.dma_start(wq_t[:, :], wq_d[:, :])
            nc.sync.dma_start(wk_t[:, :], wk_d[:, :])
            nc.sync.dma_start(wv_t[:, :], wv_d[:, :])
            nc.sync.dma_start(wg_t[:, :], wg_d[:, :])
            nc.sync.dma_start(wo_t[:, :], wo_d[:, :])
            nc.sync.dma_start(expb_t[:, :], expb_d[:, :])
            nc.sync.dma_start(em_t[:, :], em_d[:, :])
            nc.sync.dma_start(bgc_t[:, :], bgc_d[:, :])
            nc.sync.dma_start(bo_t[:, :], bo_d[:, :])

            # per-pair tiles passed from the load/proj stage to attention
            stash = {}

            def emit_load_proj(p):
                xx = wp.tile([128, 2048], f16, tag="xx")
                for s01 in range(2):
                    nc.sync.dma_start(
                        xx[:, s01 * 1024:(s01 + 1) * 1024].rearrange(
                            "p (cc q) -> p cc q", cc=4),
                        x_d[2 * p + s01].rearrange("(cc p) q -> p cc q", p=128))
                x4 = xx.rearrange("p (s cc q) -> p s cc q", s=2, cc=4)

                # paired projections: rhs = [x_s0 | x_s1] per c-chunk, N=512
                def proj_T(w_t, bcc):
                    ps = pp.tile([128, 1024], f32, tag="pp")
                    for tcc in range(2):
                        for cc in range(2):
                            nc.tensor.matmul(
                                ps[:, tcc * 512:(tcc + 1) * 512].rearrange(
                                    "p (s q) -> p s q", s=2),
                                w_t[:, cc * 256 + tcc * 128:
                                    cc * 256 + tcc * 128 + 128],
                                x4[:, :, bcc + cc, :],
                                start=(cc == 0), stop=(cc == 1))
                    return ps

                qt_ps = proj_T(wq_t, 0)
                qt = wp.tile([128, 1024], f16, tag="qt")
                nc.vector.tensor_copy(qt[:, :], qt_ps[:, :])

                kt_ps = proj_T(wk_t, 2)
                kt = wp.tile([128, 1024], f16, tag="kt")
                nc.vector.tensor_copy(kt[:, :], kt_ps[:, :])

                g_ps = proj_T(wg_t, 0)
                gs = wp.tile([128, 1024], f16, tag="gs")
                for tcc in range(2):
                    nc.scalar.activation(
                        gs[:, tcc * 512:(tcc + 1) * 512],
                        g_ps[:, tcc * 512:(tcc + 1) * 512],
                        AF.Tanh, scale=0.5,
                        bias=bgc_t[:, tcc:tcc + 1])

                # v natural per row; v' = v * exp(mask)[k]
                v_ps = pp.tile([128, 1024], f32, tag="pp")
                for s01 in range(2):
                    for kc in range(2):
                        for cc in range(2):
                            nc.tensor.matmul(
                                v_ps[:, s01 * 512 + kc * 256:
                                     s01 * 512 + kc * 256 + 256],
                                xx[:, s01 * 1024 + 512 + cc * 256 + kc * 128:
                                   s01 * 1024 + 512 + cc * 256 + kc * 128 + 128],
                                wv_t[:, cc * 256:(cc + 1) * 256],
                                start=(cc == 0), stop=(cc == 1))
                vs = wp.tile([128, 1024], bf16, tag="vs")
                for s01 in range(2):
                    s = 2 * p + s01
                    for kc in range(2):
                        nc.vector.scalar_tensor_tensor(
                            vs[:, s01 * 512 + kc * 256:
                               s01 * 512 + kc * 256 + 256],
                            v_ps[:, s01 * 512 + kc * 256:
                                 s01 * 512 + kc * 256 + 256], 1.0,
                            em_t[:, s * 64 + kc * 32:s * 64 + kc * 32 + 1
                                 ].broadcast_to((128, 256)),
                            op0=ALU.mult, op1=ALU.mult)
                stash[p] = (qt, kt, gs, vs)

            def emit_attention(p):
                qt, kt, gs, vs = stash.pop(p)
                f_ps = pp.tile([128, 1024], f32, tag="pp")
                out_sb = wp.tile([128, 1024], f32, tag="out")
                for s01 in range(2):
                    s = 2 * p + s01
                    og = wp4.tile([128, 512], f16, tag="og")
                    ovz = pp.tile([128, 1024], f32, tag="pp")
                    for hg in range(2):
                        scA = pscA.tile([128, 1024], f32, tag="scA")
                        scB = pscB.tile([128, 1024], f32, tag="scB")
                        for kc in range(2):
                            for hh in range(4):
                                tgt = scA if hh < 2 else scB
                                col = (hh % 2) * 512 + kc * 256
                                nc.tensor.matmul(
                                    tgt[:, col:col + 256],
                                    kt[32 * hh:32 * hh + 32,
                                       hg * 512 + s01 * 256 + kc * 128:
                                       hg * 512 + s01 * 256 + kc * 128 + 128],
                                    qt[32 * hh:32 * hh + 32,
                                       hg * 512 + s01 * 256:
                                       hg * 512 + s01 * 256 + 256],
                                    start=True, stop=True,
                                    tile_position=(32 * hh, 0))
                        eSa = wp4.tile([128, 1024], bf16, tag="eSa")
                        nc.scalar.activation(eSa[:, :], scA[:, :], AF.Exp)
                        eSb = wp4.tile([128, 1024], bf16, tag="eSb")
                        nc.scalar.activation(eSb[:, :], scB[:, :], AF.Exp)
                        Aa = wp4.tile([128, 1024], bf16, tag="Aa")
                        Ab = wp4.tile([128, 1024], bf16, tag="Ab")
                        mul_a = nc.gpsimd if (gp_mul and hg == 1) else nc.vector
                        mul_a.tensor_tensor(
                            Aa[:, :], eSa[:, :],
                            expb_t[:, hg * 2048:hg * 2048 + 1024],
                            op=ALU.mult)
                        nc.vector.tensor_tensor(
                            Ab[:, :], eSb[:, :],
                            expb_t[:, hg * 2048 + 1024:hg * 2048 + 2048],
                            op=ALU.mult)

                        for hh in range(4):
                            Ax = Aa if hh < 2 else Ab
                            for kc in range(2):
                                nc.tensor.matmul(
                                    ovz[32 * hh:32 * hh + 32,
                                        hg * 512:hg * 512 + 256],
                                    vs[:, s01 * 512 + kc * 256 +
                                       (hg * 4 + hh) * 32:
                                       s01 * 512 + kc * 256 +
                                       (hg * 4 + hh) * 32 + 32],
                                    Ax[:, (hh % 2) * 512 + kc * 256:
                                       (hh % 2) * 512 + kc * 256 + 256],
                                    start=(kc == 0), stop=(kc == 1),
                                    tile_position=(0, 32 * hh))
                        for hh in range(4):
                            Ax = Aa if hh < 2 else Ab
                            for kc in range(2):
                                nc.tensor.matmul(
                                    ovz[32 * hh:32 * hh + 32,
                                        hg * 512 + 256:hg * 512 + 512],
                                    em_t[:, s * 64 + kc * 32:
                                         s * 64 + kc * 32 + 32],
                                    Ax[:, (hh % 2) * 512 + kc * 256:
                                       (hh % 2) * 512 + kc * 256 + 256],
                                    start=(kc == 0), stop=(kc == 1),
                                    tile_position=(0, 32 * hh))

                        dd = wp4.tile([128, 256], f16, tag="dd")
                        if use_div:
                            nc.vector.tensor_tensor(
                                dd[:, :], ovz[:, hg * 512:hg * 512 + 256],
                                ovz[:, hg * 512 + 256:hg * 512 + 512],
                                op=ALU.divide)
                        else:
                            rz = wp4.tile([128, 256], f32, tag="rz")
                            nc.vector.reciprocal_approx_fast(
                                rz[:, :], ovz[:, hg * 512 + 256:hg * 512 + 512])
                            nc.vector.tensor_tensor(
                                dd[:, :], ovz[:, hg * 512:hg * 512 + 256],
                                rz[:, :], op=ALU.mult)
                        nc.vector.scalar_tensor_tensor(
                            og[:, hg * 256:(hg + 1) * 256],
                            gs[:, hg * 512 + s01 * 256:
                               hg * 512 + s01 * 256 + 256],
                            1.0, dd[:, :], op0=ALU.add, op1=ALU.mult)

                    # final projection for row s into the pair tile
                    for qc in range(2):
                        for tcc in range(2):
                            nc.tensor.matmul(
                                f_ps[:, s01 * 512 + qc * 256:
                                     s01 * 512 + qc * 256 + 256],
                                og[:, tcc * 256 + qc * 128:
                                   tcc * 256 + qc * 128 + 128],
                                wo_t[:, tcc * 256:(tcc + 1) * 256],
                                start=(tcc == 0), stop=(tcc == 1))
                # evict + bo for the whole pair in one DVE op
                nc.vector.tensor_tensor(
                    out_sb[:, :].rearrange("p (s qc c) -> p s qc c", s=2, qc=2),
                    f_ps[:, :].rearrange("p (s qc c) -> p s qc c", s=2, qc=2),
                    bo_t[:, :].unsqueeze(1).unsqueeze(1).broadcast_to(
                        (128, 2, 2, 256)),
                    op=ALU.add)
                for s01 in range(2):
                    nc.gpsimd.dma_start(
                        out_d[2 * p + s01].rearrange("(qc p) c -> p qc c", p=128),
                        out_sb[:, s01 * 512:(s01 + 1) * 512].rearrange(
                            "p (qc c) -> p qc c", qc=2))

            # software pipeline: proj(p) emitted before attention(p-1)
            for p in range(npair + 1):
                if p < npair:
                    emit_load_proj(p)
                if p >= 1:
                    emit_attention(p - 1)

    nc.compile()
    return nc


def get_program(s_loc=S_LOC):
    key = (s_loc, os.environ.get("KDIV", "recip"), os.environ.get("KGP", "1"))
    if key not in _CACHE:
        _CACHE[key] = _build_program(s_loc)
    return _CACHE[key]


def prep_inputs(q_x, kv_x, bias_mask, bias_pair, Wq, Wk, Wv, Wg, bg, Wo, bo,
                s_loc=S_LOC, n_cores=N_CORES):
    """Host-side layout prep. Returns per-core in_maps."""
    bf16 = ml_dtypes.bfloat16
    f16 = np.float16

    def wprep(wt):  # (in_dim, out_dim) -> [p, (cc, out)]
        return np.ascontiguousarray(
            wt.reshape(2, 128, 256).transpose(1, 0, 2).reshape(128, 512)
        ).astype(f16)

    wq_h = wprep(np.asarray(Wq).T)     # lhsT[c, t] = Wq[t, c]
    wk_h = wprep(np.asarray(Wk).T)
    wg_h = wprep(np.asarray(Wg).T)
    wv_h = wprep(np.asarray(Wv).T)     # rhs[c, t]
    wo_h = wprep(np.asarray(Wo).T * 0.5)  # rhs[t, c]; 0.5 = sigmoid fold

    bgc = np.ascontiguousarray(
        (0.5 * np.asarray(bg, np.float32)).reshape(2, 128).T)  # [128, tc]
    bo_h = np.ascontiguousarray(np.broadcast_to(
        np.asarray(bo, np.float32), (128, 256)))

    # exp(pair)^T as [128, (hg, pr, u, kc, q)], h = hg*4 + pr*2 + u
    eb = np.exp(np.asarray(bias_pair[0, 0], np.float64))  # (H, Q, K)
    ebT = eb.transpose(0, 2, 1)  # (H, K, Q)
    expb_h = np.ascontiguousarray(
        ebT.reshape(2, 2, 2, 2, 128, Q).transpose(4, 0, 1, 2, 3, 5
                                                  ).reshape(128, 4096)
    ).astype(bf16)

    x_all = np.concatenate([
        np.asarray(q_x[0], f16).transpose(0, 2, 1),
        np.asarray(kv_x[0], f16).transpose(0, 2, 1)], axis=1)
    x_all = np.ascontiguousarray(x_all)   # (S, 2C, Q): xq | xkv
    # exp(mask) replicated 32x: [128, (s, kc, 32)]
    em_all = np.exp(np.asarray(bias_mask[0, :, 0, 0, :], np.float64))  # (S, K)

    in_maps = []
    for core in range(n_cores):
        lo = core * s_loc
        em = em_all[lo:lo + s_loc].reshape(s_loc, 2, 128)  # (s, kc, p)
        em_h = np.ascontiguousarray(np.broadcast_to(
            em.transpose(2, 0, 1)[:, :, :, None], (128, s_loc, 2, 32)
        ).reshape(128, s_loc * 64)).astype(bf16)
        in_maps.append({
            "x": x_all[lo:lo + s_loc],
            "em": em_h, "expb": expb_h,
            "wq": wq_h, "wk": wk_h, "wv": wv_h, "wg": wg_h, "wo": wo_h,
            "bgc": bgc, "bo": bo_h,
        })
    return in_maps


def kernel(q_x, kv_x, bias_mask, bias_pair, Wq, Wk, Wv, Wg, bg, Wo, bo):
    from concourse import bass_utils

    nc = get_program()
    in_maps = prep_inputs(q_x, kv_x, bias_mask, bias_pair,
                          Wq, Wk, Wv, Wg, bg, Wo, bo)
    res = bass_utils.run_bass_kernel_spmd(
        nc, in_maps, core_ids=list(range(N_CORES)))
    out = np.concatenate([res.results[i]["out"] for i in range(N_CORES)], axis=0)
    return out.reshape(B, S, Q, C).astype(np.float32)


# revision 9
# speedup vs baseline: 1.5549x; 1.1778x over previous
"""Trainium2 Bass kernel for the sparse_attention nn.Module problem.

Strategy: data-parallel over the MSA-row dim S (S=128 -> 16 rows per core,
8 cores). All projection weights + pair bias replicated; activations and
mask sharded with S. No collectives.

Per-core dataflow (scheme C2 -- fully transposed attention, tile_position
packed matmuls, mask folded into v / Z so exp needs no bias, row-PAIRED
projections so the shared weights stream N=512, software-pipelined
emission: projections of pair p interleave with attention of pair p-1):
  qT/kT/gT = W @ [x_s0^T | x_s1^T]   (paired N=512 matmuls, PSUM f32,
                                      DVE evict fp16 / ACT tanh for gate;
                                      bg folded into the ACT bias)
  v'_s     = (kv_x @ Wv^T) * exp(mask)[k]    (mask folded into v rows)
  sT_h     = kT_h^T @ qT_h    (4-way ROW-packed tile_position=(32hh,0);
                               concurrent MMs drain into 4 distinct PSUM
                               banks across two 2-bank tiles scA/scB)
  expS     = exp(sT)          (no bias -> [128,1024] ACT ops)
  A        = expS * exp(pair) (DVE bf16; one chunk per row on GpSimd)
  oT_h     = v'_h^T @ A_h     (4-way COL-packed tile_position=(0,32hh))
  Zbc_h    = em^T @ A_h       (same col-packing, lhsT = exp(mask) x32
                               -> Z_h[q] lane-aligned with oT_h)
  og       = (tanh((gT+bg)/2)+1) * oT / Zbc   (0.5 folded into Wo)
  out      = og^T @ (0.5*Wo)^T + bo           (bo added in the eviction)
"""

import os
import numpy as np
import ml_dtypes

B, S, Q, C = 1, 128, 256, 256
H, DH = 8, 32
TOT = H * DH
N_CORES = 8
S_LOC = S // N_CORES  # 16

_CACHE = {}


def _build_program(s_loc):
    import concourse.bacc as bacc
    import concourse.mybir as mybir
    from concourse import tile

    dt = mybir.dt
    f32, bf16, f16 = dt.float32, dt.bfloat16, dt.float16
    AF = mybir.ActivationFunctionType
    ALU = mybir.AluOpType
    use_div = os.environ.get("KDIV", "recip") == "div"
    gp_mul = int(os.environ.get("KGP", "1"))  # A-mul chunks on GpSimd /row

    npair = s_loc // 2

    nc = bacc.Bacc("TRN2", target_bir_lowering=False, debug=False,
                   num_devices=N_CORES)

    x_d = nc.dram_tensor("x", [s_loc, 2 * C, Q], f16, kind="ExternalInput").ap()
    wq_d = nc.dram_tensor("wq", [128, 512], f16, kind="ExternalInput").ap()
    wk_d = nc.dram_tensor("wk", [128, 512], f16, kind="ExternalInput").ap()
    wv_d = nc.dram_tensor("wv", [128, 512], f16, kind="ExternalInput").ap()
    wg_d = nc.dram_tensor("wg", [128, 512], f16, kind="ExternalInput").ap()
    wo_d = nc.dram_tensor("wo", [128, 512], f16, kind="ExternalInput").ap()
    expb_d = nc.dram_tensor("expb", [128, 4096], bf16, kind="ExternalInput").ap()
    em_d = nc.dram_tensor("em", [128, s_loc * 64], bf16, kind="ExternalInput").ap()
    bgc_d = nc.dram_tensor("bgc", [128, 2], f32, kind="ExternalInput").ap()
    out_d = nc.dram_tensor("out", [s_loc, Q, C], f32, kind="ExternalOutput").ap()

    with tile.TileContext(nc) as tc:
        with (
            tc.tile_pool(name="const", bufs=1) as cp,
            tc.tile_pool(name="work", bufs=2) as wp,
            tc.tile_pool(name="work4", bufs=4) as wp4,
            tc.tile_pool(name="pp", bufs=2, space="PSUM") as pp,
            tc.tile_pool(name="sca", bufs=1, space="PSUM") as pscA,
            tc.tile_pool(name="scb", bufs=1, space="PSUM") as pscB,
        ):
            # ---- resident constants ----
            wq_t = cp.tile([128, 512], f16, tag="wq")
            wk_t = cp.tile([128, 512], f16, tag="wk")
            wv_t = cp.tile([128, 512], f16, tag="wv")
            wg_t = cp.tile([128, 512], f16, tag="wg")
            wo_t = cp.tile([128, 512], f16, tag="wo")
            expb_t = cp.tile([128, 4096], bf16, tag="expb")
            em_t = cp.tile([128, s_loc * 64], bf16, tag="em")
            bgc_t = cp.tile([128, 2], f32, tag="bgc")

            nc.sync.dma_start(wq_t[:, :], wq_d[:, :])
            nc.sync.dma_start(wk_t[:, :], wk_d[:, :])
            nc.sync.dma_start(wv_t[:, :], wv_d[:, :])
            nc.sync.dma_start(wg_t[:, :], wg_d[:, :])
            nc.sync.dma_start(wo_t[:, :], wo_d[:, :])
            nc.sync.dma_start(expb_t[:, :], expb_d[:, :])
            nc.sync.dma_start(em_t[:, :], em_d[:, :])
            nc.sync.dma_start(bgc_t[:, :], bgc_d[:, :])

            # per-pair tiles passed from the load/proj stage to attention
            stash = {}

            def emit_load_proj(p):
                xx = wp.tile([128, 2048], f16, tag="xx")
                for s01 in range(2):
                    nc.sync.dma_start(
                        xx[:, s01 * 1024:(s01 + 1) * 1024].rearrange(
                            "p (cc q) -> p cc q", cc=4),
                        x_d[2 * p + s01].rearrange("(cc p) q -> p cc q", p=128))
                x4 = xx.rearrange("p (s cc q) -> p s cc q", s=2, cc=4)

                # paired projections: rhs = [x_s0 | x_s1] per c-chunk, N=512
                def proj_T(w_t, bcc):
                    ps = pp.tile([128, 1024], f32, tag="pp")
                    for tcc in range(2):
                        for cc in range(2):
                            nc.tensor.matmul(
                                ps[:, tcc * 512:(tcc + 1) * 512].rearrange(
                                    "p (s q) -> p s q", s=2),
                                w_t[:, cc * 256 + tcc * 128:
                                    cc * 256 + tcc * 128 + 128],
                                x4[:, :, bcc + cc, :],
                                start=(cc == 0), stop=(cc == 1))
                    return ps

                qt_ps = proj_T(wq_t, 0)
                qt = wp.tile([128, 1024], f16, tag="qt")
                nc.vector.tensor_copy(qt[:, :], qt_ps[:, :])

                kt_ps = proj_T(wk_t, 2)
                kt = wp.tile([128, 1024], f16, tag="kt")
                nc.vector.tensor_copy(kt[:, :], kt_ps[:, :])

                g_ps = proj_T(wg_t, 0)
                gs = wp.tile([128, 1024], f16, tag="gs")
                for tcc in range(2):
                    nc.scalar.activation(
                        gs[:, tcc * 512:(tcc + 1) * 512],
                        g_ps[:, tcc * 512:(tcc + 1) * 512],
                        AF.Tanh, scale=0.5,
                        bias=bgc_t[:, tcc:tcc + 1])

                # v natural per row; v' = v * exp(mask)[k]
                v_ps = pp.tile([128, 1024], f32, tag="pp")
                for s01 in range(2):
                    for kc in range(2):
                        for cc in range(2):
                            nc.tensor.matmul(
                                v_ps[:, s01 * 512 + kc * 256:
                                     s01 * 512 + kc * 256 + 256],
                                xx[:, s01 * 1024 + 512 + cc * 256 + kc * 128:
                                   s01 * 1024 + 512 + cc * 256 + kc * 128 + 128],
                                wv_t[:, cc * 256:(cc + 1) * 256],
                                start=(cc == 0), stop=(cc == 1))
                vs = wp.tile([128, 1024], bf16, tag="vs")
                for s01 in range(2):
                    s = 2 * p + s01
                    nc.vector.scalar_tensor_tensor(
                        vs[:, s01 * 512:(s01 + 1) * 512].rearrange(
                            "p (kc t) -> p kc t", kc=2),
                        v_ps[:, s01 * 512:(s01 + 1) * 512].rearrange(
                            "p (kc t) -> p kc t", kc=2), 1.0,
                        em_t[:, s * 64:(s + 1) * 64].rearrange(
                            "p (kc e) -> p kc e", kc=2)[:, :, 0:1
                            ].broadcast_to((128, 2, 256)),
                        op0=ALU.mult, op1=ALU.mult)
                stash[p] = (qt, kt, gs, vs)

            def emit_attention(p):
                qt, kt, gs, vs = stash.pop(p)
                ogs = []
                for s01 in range(2):
                    s = 2 * p + s01
                    og = wp4.tile([128, 512], f16, tag="og")
                    ogs.append(og)
                    ovz = pp.tile([128, 1024], f32, tag="pp")
                    for hg in range(2):
                        scA = pscA.tile([128, 1024], f32, tag="scA")
                        scB = pscB.tile([128, 1024], f32, tag="scB")
                        for kc in range(2):
                            for hh in range(4):
                                tgt = scA if hh < 2 else scB
                                col = (hh % 2) * 512 + kc * 256
                                nc.tensor.matmul(
                                    tgt[:, col:col + 256],
                                    kt[32 * hh:32 * hh + 32,
                                       hg * 512 + s01 * 256 + kc * 128:
                                       hg * 512 + s01 * 256 + kc * 128 + 128],
                                    qt[32 * hh:32 * hh + 32,
                                       hg * 512 + s01 * 256:
                                       hg * 512 + s01 * 256 + 256],
                                    start=True, stop=True,
                                    tile_position=(32 * hh, 0))
                        eSa = wp4.tile([128, 1024], bf16, tag="eSa")
                        nc.scalar.activation(eSa[:, :], scA[:, :], AF.Exp)
                        eSb = wp4.tile([128, 1024], bf16, tag="eSb")
                        nc.scalar.activation(eSb[:, :], scB[:, :], AF.Exp)
                        Aa = wp4.tile([128, 1024], bf16, tag="Aa")
                        Ab = wp4.tile([128, 1024], bf16, tag="Ab")
                        eng = nc.gpsimd if (gp_mul and hg == 1) else nc.vector
                        eng.tensor_tensor(
                            Aa[:, :], eSa[:, :],
                            expb_t[:, hg * 2048:hg * 2048 + 1024],
                            op=ALU.mult)
                        eng.tensor_tensor(
                            Ab[:, :], eSb[:, :],
                            expb_t[:, hg * 2048 + 1024:hg * 2048 + 2048],
                            op=ALU.mult)

                        for hh in range(4):
                            Ax = Aa if hh < 2 else Ab
                            for kc in range(2):
                                nc.tensor.matmul(
                                    ovz[32 * hh:32 * hh + 32,
                                        hg * 512:hg * 512 + 256],
                                    vs[:, s01 * 512 + kc * 256 +
                                       (hg * 4 + hh) * 32:
                                       s01 * 512 + kc * 256 +
                                       (hg * 4 + hh) * 32 + 32],
                                    Ax[:, (hh % 2) * 512 + kc * 256:
                                       (hh % 2) * 512 + kc * 256 + 256],
                                    start=(kc == 0), stop=(kc == 1),
                                    tile_position=(0, 32 * hh))
                        for hh in range(4):
                            Ax = Aa if hh < 2 else Ab
                            for kc in range(2):
                                nc.tensor.matmul(
                                    ovz[32 * hh:32 * hh + 32,
                                        hg * 512 + 256:hg * 512 + 512],
                                    em_t[:, s * 64 + kc * 32:
                                         s * 64 + kc * 32 + 32],
                                    Ax[:, (hh % 2) * 512 + kc * 256:
                                       (hh % 2) * 512 + kc * 256 + 256],
                                    start=(kc == 0), stop=(kc == 1),
                                    tile_position=(0, 32 * hh))

                        dd = wp4.tile([128, 256], f16, tag="dd")
                        if use_div:
                            nc.vector.tensor_tensor(
                                dd[:, :], ovz[:, hg * 512:hg * 512 + 256],
                                ovz[:, hg * 512 + 256:hg * 512 + 512],
                                op=ALU.divide)
                        else:
                            rz = wp4.tile([128, 256], f32, tag="rz")
                            nc.vector.reciprocal_approx_fast(
                                rz[:, :], ovz[:, hg * 512 + 256:hg * 512 + 512])
                            nc.vector.tensor_tensor(
                                dd[:, :], ovz[:, hg * 512:hg * 512 + 256],
                                rz[:, :], op=ALU.mult)
                        nc.vector.scalar_tensor_tensor(
                            og[:, hg * 256:(hg + 1) * 256],
                            gs[:, hg * 512 + s01 * 256:
                               hg * 512 + s01 * 256 + 256],
                            1.0, dd[:, :], op0=ALU.add, op1=ALU.mult)

                # final projection for both rows after all scA uses
                f_ps = pscA.tile([128, 1024], f32, tag="scA", name="f_ps")
                out_sb = wp.tile([128, 1024], f32, tag="out")
                for s01 in range(2):
                    og = ogs[s01]
                    for qc in range(2):
                        for tcc in range(2):
                            nc.tensor.matmul(
                                f_ps[:, s01 * 512 + qc * 256:
                                     s01 * 512 + qc * 256 + 256],
                                og[:, tcc * 256 + qc * 128:
                                   tcc * 256 + qc * 128 + 128],
                                wo_t[:, tcc * 256:(tcc + 1) * 256],
                                start=(tcc == 0), stop=(tcc == 1))
                # evict the whole pair in one DVE op (bo added on host)
                nc.vector.tensor_copy(out_sb[:, :], f_ps[:, :])
                for s01 in range(2):
                    nc.sync.dma_start(
                        out_d[2 * p + s01].rearrange("(qc p) c -> p qc c", p=128),
                        out_sb[:, s01 * 512:(s01 + 1) * 512].rearrange(
                            "p (qc c) -> p qc c", qc=2))

            # software pipeline: proj(p) emitted before attention(p-1)
            for p in range(npair + 1):
                if p < npair:
                    emit_load_proj(p)
                if p >= 1:
                    emit_attention(p - 1)

    nc.compile()
    return nc


def get_program(s_loc=S_LOC):
    key = (s_loc, os.environ.get("KDIV", "recip"), os.environ.get("KGP", "1"))
    if key not in _CACHE:
        _CACHE[key] = _build_program(s_loc)
    return _CACHE[key]


def prep_inputs(q_x, kv_x, bias_mask, bias_pair, Wq, Wk, Wv, Wg, bg, Wo, bo,
                s_loc=S_LOC, n_cores=N_CORES):
    """Host-side layout prep. Returns per-core in_maps."""
    bf16 = ml_dtypes.bfloat16
    f16 = np.float16

    def wprep(wt):  # (in_dim, out_dim) -> [p, (cc, out)]
        return np.ascontiguousarray(
            wt.reshape(2, 128, 256).transpose(1, 0, 2).reshape(128, 512)
        ).astype(f16)

    wq_h = wprep(np.asarray(Wq).T)     # lhsT[c, t] = Wq[t, c]
    wk_h = wprep(np.asarray(Wk).T)
    wg_h = wprep(np.asarray(Wg).T)
    wv_h = wprep(np.asarray(Wv).T)     # rhs[c, t]
    wo_h = wprep(np.asarray(Wo).T * 0.5)  # rhs[t, c]; 0.5 = sigmoid fold

    bgc = np.ascontiguousarray(
        (0.5 * np.asarray(bg, np.float32)).reshape(2, 128).T)  # [128, tc]

    # exp(pair)^T as [128, (hg, pr, u, kc, q)], h = hg*4 + pr*2 + u
    eb = np.exp(np.asarray(bias_pair[0, 0], np.float64))  # (H, Q, K)
    ebT = eb.transpose(0, 2, 1)  # (H, K, Q)
    expb_h = np.ascontiguousarray(
        ebT.reshape(2, 2, 2, 2, 128, Q).transpose(4, 0, 1, 2, 3, 5
                                                  ).reshape(128, 4096)
    ).astype(bf16)

    x_all = np.concatenate([
        np.asarray(q_x[0], f16).transpose(0, 2, 1),
        np.asarray(kv_x[0], f16).transpose(0, 2, 1)], axis=1)
    x_all = np.ascontiguousarray(x_all)   # (S, 2C, Q): xq | xkv
    # exp(mask) replicated 32x: [128, (s, kc, 32)]
    em_all = np.exp(np.asarray(bias_mask[0, :, 0, 0, :], np.float64))  # (S, K)

    in_maps = []
    for core in range(n_cores):
        lo = core * s_loc
        em = em_all[lo:lo + s_loc].reshape(s_loc, 2, 128)  # (s, kc, p)
        em_h = np.ascontiguousarray(np.broadcast_to(
            em.transpose(2, 0, 1)[:, :, :, None], (128, s_loc, 2, 32)
        ).reshape(128, s_loc * 64)).astype(bf16)
        in_maps.append({
            "x": x_all[lo:lo + s_loc],
            "em": em_h, "expb": expb_h,
            "wq": wq_h, "wk": wk_h, "wv": wv_h, "wg": wg_h, "wo": wo_h,
            "bgc": bgc,
        })
    return in_maps


def kernel(q_x, kv_x, bias_mask, bias_pair, Wq, Wk, Wv, Wg, bg, Wo, bo):
    from concourse import bass_utils

    nc = get_program()
    in_maps = prep_inputs(q_x, kv_x, bias_mask, bias_pair,
                          Wq, Wk, Wv, Wg, bg, Wo, bo)
    res = bass_utils.run_bass_kernel_spmd(
        nc, in_maps, core_ids=list(range(N_CORES)))
    out = np.concatenate([res.results[i]["out"] for i in range(N_CORES)], axis=0)
    out = out.reshape(B, S, Q, C).astype(np.float32)
    return out + np.asarray(bo, np.float32)


# revision 12
# speedup vs baseline: 1.6238x; 1.0443x over previous
"""Trainium2 Bass kernel for the sparse_attention nn.Module problem.

Strategy: data-parallel over the MSA-row dim S (S=128 -> 16 rows per core,
8 cores). All projection weights + pair bias replicated; activations and
mask sharded with S. No collectives.

Per-core dataflow (scheme C2 -- fully transposed attention, tile_position
packed matmuls, mask folded into v / Z so exp needs no bias, row-PAIRED
projections so the shared weights stream N=512, software-pipelined
emission: projections of pair p interleave with attention of pair p-1):
  qT/kT/gT = W @ [x_s0^T | x_s1^T]   (paired N=512 matmuls, PSUM f32,
                                      DVE evict fp16 / ACT tanh for gate;
                                      bg folded into the ACT bias)
  v'_s     = (kv_x @ Wv^T) * exp(mask)[k]    (mask folded into v rows)
  sT_h     = kT_h^T @ qT_h    (4-way ROW-packed tile_position=(32hh,0);
                               concurrent MMs drain into 4 distinct PSUM
                               banks across two 2-bank tiles scA/scB)
  expS     = exp(sT)          (no bias -> [128,1024] ACT ops)
  A        = expS * exp(pair) (DVE bf16; one chunk per row on GpSimd)
  oT_h     = v'_h^T @ A_h     (4-way COL-packed tile_position=(0,32hh))
  Zbc_h    = em^T @ A_h       (same col-packing, lhsT = exp(mask) x32
                               -> Z_h[q] lane-aligned with oT_h)
  og       = (tanh((gT+bg)/2)+1) * oT / Zbc   (0.5 folded into Wo)
  out      = og^T @ (0.5*Wo)^T + bo           (bo added in the eviction)
"""

import os
import numpy as np
import ml_dtypes

B, S, Q, C = 1, 128, 256, 256
H, DH = 8, 32
TOT = H * DH
N_CORES = 8
S_LOC = S // N_CORES  # 16

_CACHE = {}


def _build_program(s_loc):
    import concourse.bacc as bacc
    import concourse.mybir as mybir
    from concourse import tile

    dt = mybir.dt
    f32, bf16, f16 = dt.float32, dt.bfloat16, dt.float16
    AF = mybir.ActivationFunctionType
    ALU = mybir.AluOpType
    use_div = os.environ.get("KDIV", "recip") == "div"
    gp_mul = int(os.environ.get("KGP", "1"))  # A-mul chunks on GpSimd /row

    npair = s_loc // 2

    nc = bacc.Bacc("TRN2", target_bir_lowering=False, debug=False,
                   num_devices=N_CORES)

    x_d = nc.dram_tensor("x", [s_loc, 2 * C, Q], f16, kind="ExternalInput").ap()
    wq_d = nc.dram_tensor("wq", [128, 512], f16, kind="ExternalInput").ap()
    wk_d = nc.dram_tensor("wk", [128, 512], f16, kind="ExternalInput").ap()
    wv_d = nc.dram_tensor("wv", [128, 512], f16, kind="ExternalInput").ap()
    wg_d = nc.dram_tensor("wg", [128, 512], f16, kind="ExternalInput").ap()
    wo_d = nc.dram_tensor("wo", [128, 512], f16, kind="ExternalInput").ap()
    expb_d = nc.dram_tensor("expb", [128, 4096], bf16, kind="ExternalInput").ap()
    em_d = nc.dram_tensor("em", [128, s_loc * 64], bf16, kind="ExternalInput").ap()
    bgc_d = nc.dram_tensor("bgc", [128, 2], f32, kind="ExternalInput").ap()
    id_d = nc.dram_tensor("ident", [128, 128], bf16, kind="ExternalInput").ap()
    out_d = nc.dram_tensor("out", [s_loc, Q, C], f32, kind="ExternalOutput").ap()

    with tile.TileContext(nc) as tc:
        with (
            tc.tile_pool(name="const", bufs=1) as cp,
            tc.tile_pool(name="work", bufs=2) as wp,
            tc.tile_pool(name="work4", bufs=4) as wp4,
            tc.tile_pool(name="pp", bufs=2, space="PSUM") as pp,
            tc.tile_pool(name="sca", bufs=1, space="PSUM") as pscA,
            tc.tile_pool(name="scb", bufs=1, space="PSUM") as pscB,
        ):
            # ---- resident constants ----
            wq_t = cp.tile([128, 512], f16, tag="wq")
            wk_t = cp.tile([128, 512], f16, tag="wk")
            wv_t = cp.tile([128, 512], f16, tag="wv")
            wg_t = cp.tile([128, 512], f16, tag="wg")
            wo_t = cp.tile([128, 512], f16, tag="wo")
            expb_t = cp.tile([128, 4096], bf16, tag="expb")
            em_t = cp.tile([128, s_loc * 64], bf16, tag="em")
            bgc_t = cp.tile([128, 2], f32, tag="bgc")
            id_t = cp.tile([128, 128], bf16, tag="ident")

            nc.sync.dma_start(wq_t[:, :], wq_d[:, :])
            nc.sync.dma_start(wk_t[:, :], wk_d[:, :])
            nc.sync.dma_start(wv_t[:, :], wv_d[:, :])
            nc.sync.dma_start(wg_t[:, :], wg_d[:, :])
            nc.sync.dma_start(wo_t[:, :], wo_d[:, :])
            nc.sync.dma_start(expb_t[:, :], expb_d[:, :])
            nc.sync.dma_start(em_t[:, :], em_d[:, :])
            nc.sync.dma_start(bgc_t[:, :], bgc_d[:, :])
            nc.sync.dma_start(id_t[:, :], id_d[:, :])

            # per-pair tiles passed from the load/proj stage to attention
            stash = {}

            def emit_load_proj(p):
                xx = wp.tile([128, 2048], f16, tag="xx")
                for s01 in range(2):
                    nc.sync.dma_start(
                        xx[:, s01 * 1024:(s01 + 1) * 1024].rearrange(
                            "p (cc q) -> p cc q", cc=4),
                        x_d[2 * p + s01].rearrange("(cc p) q -> p cc q", p=128))
                x4 = xx.rearrange("p (s cc q) -> p s cc q", s=2, cc=4)

                # paired projections: rhs = [x_s0 | x_s1] per c-chunk, N=512
                def proj_T(w_t, bcc):
                    ps = pp.tile([128, 1024], f32, tag="pp")
                    for tcc in range(2):
                        for cc in range(2):
                            nc.tensor.matmul(
                                ps[:, tcc * 512:(tcc + 1) * 512].rearrange(
                                    "p (s q) -> p s q", s=2),
                                w_t[:, cc * 256 + tcc * 128:
                                    cc * 256 + tcc * 128 + 128],
                                x4[:, :, bcc + cc, :],
                                start=(cc == 0), stop=(cc == 1))
                    return ps

                qt_ps = proj_T(wq_t, 0)
                qt = wp.tile([128, 1024], f16, tag="qt")
                nc.vector.tensor_copy(qt[:, :], qt_ps[:, :])

                kt_ps = proj_T(wk_t, 2)
                kt = wp.tile([128, 1024], f16, tag="kt")
                nc.vector.tensor_copy(kt[:, :], kt_ps[:, :])

                g_ps = proj_T(wg_t, 0)
                gs0 = wp.tile([128, 1024], f16, tag="gs0")
                for tcc in range(2):
                    nc.scalar.activation(
                        gs0[:, tcc * 512:(tcc + 1) * 512],
                        g_ps[:, tcc * 512:(tcc + 1) * 512],
                        AF.Tanh, scale=0.5,
                        bias=bgc_t[:, tcc:tcc + 1])
                # gs = tanh(.)+1 so the gate apply is a plain 2-input mult
                gs = wp.tile([128, 1024], f16, tag="gs")
                nc.vector.tensor_scalar(
                    gs[:, :], gs0[:, :], 1.0, None, op0=ALU.add)

                # v natural per row; v' = v * exp(mask)[k]
                v_ps = pp.tile([128, 1024], f32, tag="pp")
                for s01 in range(2):
                    for kc in range(2):
                        for cc in range(2):
                            nc.tensor.matmul(
                                v_ps[:, s01 * 512 + kc * 256:
                                     s01 * 512 + kc * 256 + 256],
                                xx[:, s01 * 1024 + 512 + cc * 256 + kc * 128:
                                   s01 * 1024 + 512 + cc * 256 + kc * 128 + 128],
                                wv_t[:, cc * 256:(cc + 1) * 256],
                                start=(cc == 0), stop=(cc == 1))
                vs = wp.tile([128, 1024], bf16, tag="vs")
                for s01 in range(2):
                    s = 2 * p + s01
                    nc.vector.scalar_tensor_tensor(
                        vs[:, s01 * 512:(s01 + 1) * 512].rearrange(
                            "p (kc t) -> p kc t", kc=2),
                        v_ps[:, s01 * 512:(s01 + 1) * 512].rearrange(
                            "p (kc t) -> p kc t", kc=2), 1.0,
                        em_t[:, s * 64:(s + 1) * 64].rearrange(
                            "p (kc e) -> p kc e", kc=2)[:, :, 0:1
                            ].broadcast_to((128, 2, 256)),
                        op0=ALU.mult, op1=ALU.mult)
                stash[p] = (qt, kt, gs, vs)

            def emit_attention(p):
                qt, kt, gs, vs = stash.pop(p)
                ogs = []
                for s01 in range(2):
                    s = 2 * p + s01
                    og = wp4.tile([128, 512], f16, tag="og")
                    ogs.append(og)
                    ovz = pp.tile([128, 1024], f32, tag="pp")
                    for hg in range(2):
                        scA = pscA.tile([128, 1024], f32, tag="scA")
                        scB = pscB.tile([128, 1024], f32, tag="scB")
                        # seed PSUM with the pair bias (identity matmul)
                        for tgt, pr in ((scA, 0), (scB, 1)):
                            for b in range(2):
                                nc.tensor.matmul(
                                    tgt[:, b * 512:(b + 1) * 512],
                                    id_t[:, :],
                                    expb_t[:, hg * 2048 + pr * 1024 + b * 512:
                                           hg * 2048 + pr * 1024 + b * 512 + 512],
                                    start=True, stop=False,
                                    skip_group_check=True)
                        for kc in range(2):
                            for hh in range(4):
                                tgt = scA if hh < 2 else scB
                                col = (hh % 2) * 512 + kc * 256
                                nc.tensor.matmul(
                                    tgt[:, col:col + 256],
                                    kt[32 * hh:32 * hh + 32,
                                       hg * 512 + s01 * 256 + kc * 128:
                                       hg * 512 + s01 * 256 + kc * 128 + 128],
                                    qt[32 * hh:32 * hh + 32,
                                       hg * 512 + s01 * 256:
                                       hg * 512 + s01 * 256 + 256],
                                    start=False, stop=(kc == 1),
                                    tile_position=(32 * hh, 0),
                                    skip_group_check=True)
                        Aa = wp4.tile([128, 1024], bf16, tag="Aa")
                        nc.scalar.activation(Aa[:, :], scA[:, :], AF.Exp)
                        Ab = wp4.tile([128, 1024], bf16, tag="Ab")
                        nc.scalar.activation(Ab[:, :], scB[:, :], AF.Exp)

                        for hh in range(4):
                            Ax = Aa if hh < 2 else Ab
                            for kc in range(2):
                                nc.tensor.matmul(
                                    ovz[32 * hh:32 * hh + 32,
                                        hg * 512:hg * 512 + 256],
                                    vs[:, s01 * 512 + kc * 256 +
                                       (hg * 4 + hh) * 32:
                                       s01 * 512 + kc * 256 +
                                       (hg * 4 + hh) * 32 + 32],
                                    Ax[:, (hh % 2) * 512 + kc * 256:
                                       (hh % 2) * 512 + kc * 256 + 256],
                                    start=(kc == 0), stop=(kc == 1),
                                    tile_position=(0, 32 * hh))
                        for hh in range(4):
                            Ax = Aa if hh < 2 else Ab
                            for kc in range(2):
                                nc.tensor.matmul(
                                    ovz[32 * hh:32 * hh + 32,
                                        hg * 512 + 256:hg * 512 + 512],
                                    em_t[:, s * 64 + kc * 32:
                                         s * 64 + kc * 32 + 32],
                                    Ax[:, (hh % 2) * 512 + kc * 256:
                                       (hh % 2) * 512 + kc * 256 + 256],
                                    start=(kc == 0), stop=(kc == 1),
                                    tile_position=(0, 32 * hh))

                        dd = wp4.tile([128, 256], f16, tag="dd")
                        if use_div:
                            nc.vector.tensor_tensor(
                                dd[:, :], ovz[:, hg * 512:hg * 512 + 256],
                                ovz[:, hg * 512 + 256:hg * 512 + 512],
                                op=ALU.divide)
                        else:
                            rz = wp4.tile([128, 256], f32, tag="rz")
                            nc.vector.reciprocal_approx_fast(
                                rz[:, :], ovz[:, hg * 512 + 256:hg * 512 + 512])
                            nc.vector.tensor_tensor(
                                dd[:, :], ovz[:, hg * 512:hg * 512 + 256],
                                rz[:, :], op=ALU.mult)
                        eng_og = nc.gpsimd if gp_mul else nc.vector
                        eng_og.tensor_tensor(
                            og[:, hg * 256:(hg + 1) * 256],
                            gs[:, hg * 512 + s01 * 256:
                               hg * 512 + s01 * 256 + 256],
                            dd[:, :], op=ALU.mult)

                # final projection for both rows after all scA uses
                f_ps = pscA.tile([128, 1024], f32, tag="scA", name="f_ps")
                out_sb = wp.tile([128, 1024], f32, tag="out")
                for s01 in range(2):
                    og = ogs[s01]
                    for qc in range(2):
                        for tcc in range(2):
                            nc.tensor.matmul(
                                f_ps[:, s01 * 512 + qc * 256:
                                     s01 * 512 + qc * 256 + 256],
                                og[:, tcc * 256 + qc * 128:
                                   tcc * 256 + qc * 128 + 128],
                                wo_t[:, tcc * 256:(tcc + 1) * 256],
                                start=(tcc == 0), stop=(tcc == 1))
                # evict the whole pair in one DVE op (bo added on host)
                nc.vector.tensor_copy(out_sb[:, :], f_ps[:, :])
                for s01 in range(2):
                    nc.sync.dma_start(
                        out_d[2 * p + s01].rearrange("(qc p) c -> p qc c", p=128),
                        out_sb[:, s01 * 512:(s01 + 1) * 512].rearrange(
                            "p (qc c) -> p qc c", qc=2))

            # software pipeline: proj(p) emitted before attention(p-1)
            for p in range(npair + 1):
                if p < npair:
                    emit_load_proj(p)
                if p >= 1:
                    emit_attention(p - 1)

    nc.compile()
    return nc


def get_program(s_loc=S_LOC):
    key = (s_loc, os.environ.get("KDIV", "recip"), os.environ.get("KGP", "1"))
    if key not in _CACHE:
        _CACHE[key] = _build_program(s_loc)
    return _CACHE[key]


def prep_inputs(q_x, kv_x, bias_mask, bias_pair, Wq, Wk, Wv, Wg, bg, Wo, bo,
                s_loc=S_LOC, n_cores=N_CORES):
    """Host-side layout prep. Returns per-core in_maps."""
    bf16 = ml_dtypes.bfloat16
    f16 = np.float16

    def wprep(wt):  # (in_dim, out_dim) -> [p, (cc, out)]
        return np.ascontiguousarray(
            wt.reshape(2, 128, 256).transpose(1, 0, 2).reshape(128, 512)
        ).astype(f16)

    wq_h = wprep(np.asarray(Wq).T)     # lhsT[c, t] = Wq[t, c]
    wk_h = wprep(np.asarray(Wk).T)
    wg_h = wprep(np.asarray(Wg).T)
    wv_h = wprep(np.asarray(Wv).T)     # rhs[c, t]
    wo_h = wprep(np.asarray(Wo).T * 0.5)  # rhs[t, c]; 0.5 = sigmoid fold

    bgc = np.ascontiguousarray(
        (0.5 * np.asarray(bg, np.float32)).reshape(2, 128).T)  # [128, tc]

    # pair^T as [128, (hg, pr, u, kc, q)], h = hg*4 + pr*2 + u
    eb = np.asarray(bias_pair[0, 0], np.float64)  # (H, Q, K)
    ebT = eb.transpose(0, 2, 1)  # (H, K, Q)
    expb_h = np.ascontiguousarray(
        ebT.reshape(2, 2, 2, 2, 128, Q).transpose(4, 0, 1, 2, 3, 5
                                                  ).reshape(128, 4096)
    ).astype(bf16)

    x_all = np.concatenate([
        np.asarray(q_x[0], f16).transpose(0, 2, 1),
        np.asarray(kv_x[0], f16).transpose(0, 2, 1)], axis=1)
    x_all = np.ascontiguousarray(x_all)   # (S, 2C, Q): xq | xkv
    # exp(mask) replicated 32x: [128, (s, kc, 32)]
    em_all = np.exp(np.asarray(bias_mask[0, :, 0, 0, :], np.float64))  # (S, K)

    in_maps = []
    for core in range(n_cores):
        lo = core * s_loc
        em = em_all[lo:lo + s_loc].reshape(s_loc, 2, 128)  # (s, kc, p)
        em_h = np.ascontiguousarray(np.broadcast_to(
            em.transpose(2, 0, 1)[:, :, :, None], (128, s_loc, 2, 32)
        ).reshape(128, s_loc * 64)).astype(bf16)
        in_maps.append({
            "x": x_all[lo:lo + s_loc],
            "em": em_h, "expb": expb_h,
            "wq": wq_h, "wk": wk_h, "wv": wv_h, "wg": wg_h, "wo": wo_h,
            "bgc": bgc, "ident": np.eye(128, dtype=bf16),
        })
    return in_maps


def kernel(q_x, kv_x, bias_mask, bias_pair, Wq, Wk, Wv, Wg, bg, Wo, bo):
    from concourse import bass_utils

    nc = get_program()
    in_maps = prep_inputs(q_x, kv_x, bias_mask, bias_pair,
                          Wq, Wk, Wv, Wg, bg, Wo, bo)
    res = bass_utils.run_bass_kernel_spmd(
        nc, in_maps, core_ids=list(range(N_CORES)))
    out = np.concatenate([res.results[i]["out"] for i in range(N_CORES)], axis=0)
    out = out.reshape(B, S, Q, C).astype(np.float32)
    return out + np.asarray(bo, np.float32)


# revision 13
# speedup vs baseline: 1.8381x; 1.1320x over previous
"""Trainium2 Bass kernel for the sparse_attention nn.Module problem.

Strategy: data-parallel over the MSA-row dim S (S=128 -> 16 rows per core,
8 cores). All projection weights + pair bias replicated; activations and
mask sharded with S. No collectives.

Per-core dataflow (scheme C2 -- fully transposed attention, tile_position
packed matmuls, mask folded into v / Z so exp needs no bias, row-PAIRED
projections so the shared weights stream N=512, software-pipelined
emission: projections of pair p interleave with attention of pair p-1):
  qT/kT/gT = W @ [x_s0^T | x_s1^T]   (paired N=512 matmuls, PSUM f32,
                                      DVE evict fp16 / ACT tanh for gate;
                                      bg folded into the ACT bias)
  v'_s     = (kv_x @ Wv^T) * exp(mask)[k]    (mask folded into v rows)
  sT_h     = kT_h^T @ qT_h    (4-way ROW-packed tile_position=(32hh,0);
                               concurrent MMs drain into 4 distinct PSUM
                               banks across two 2-bank tiles scA/scB)
  expS     = exp(sT)          (no bias -> [128,1024] ACT ops)
  A        = expS * exp(pair) (DVE bf16; one chunk per row on GpSimd)
  oT_h     = v'_h^T @ A_h     (4-way COL-packed tile_position=(0,32hh))
  Zbc_h    = em^T @ A_h       (same col-packing, lhsT = exp(mask) x32
                               -> Z_h[q] lane-aligned with oT_h)
  og       = (tanh((gT+bg)/2)+1) * oT / Zbc   (0.5 folded into Wo)
  out      = og^T @ (0.5*Wo)^T + bo           (bo added in the eviction)
"""

import os
import numpy as np
import ml_dtypes

B, S, Q, C = 1, 128, 256, 256
H, DH = 8, 32
TOT = H * DH
N_CORES = 8
S_LOC = S // N_CORES  # 16

_CACHE = {}


def _build_program(s_loc):
    import concourse.bacc as bacc
    import concourse.mybir as mybir
    from concourse import tile

    dt = mybir.dt
    f32, bf16, f16 = dt.float32, dt.bfloat16, dt.float16
    AF = mybir.ActivationFunctionType
    ALU = mybir.AluOpType
    use_div = os.environ.get("KDIV", "recip") == "div"
    gp_mul = int(os.environ.get("KGP", "1"))  # A-mul chunks on GpSimd /row

    npair = s_loc // 2

    nc = bacc.Bacc("TRN2", target_bir_lowering=False, debug=False,
                   num_devices=N_CORES)

    x_d = nc.dram_tensor("x", [s_loc, 2 * C, Q], f16, kind="ExternalInput").ap()
    wq_d = nc.dram_tensor("wq", [128, 512], f16, kind="ExternalInput").ap()
    wk_d = nc.dram_tensor("wk", [128, 512], f16, kind="ExternalInput").ap()
    wv_d = nc.dram_tensor("wv", [128, 512], f16, kind="ExternalInput").ap()
    wg_d = nc.dram_tensor("wg", [128, 512], f16, kind="ExternalInput").ap()
    wo_d = nc.dram_tensor("wo", [128, 512], f16, kind="ExternalInput").ap()
    expb_d = nc.dram_tensor("expb", [128, 4096], bf16, kind="ExternalInput").ap()
    em_d = nc.dram_tensor("em", [128, s_loc * 64], bf16, kind="ExternalInput").ap()
    bgc_d = nc.dram_tensor("bgc", [128, 2], f32, kind="ExternalInput").ap()
    id_d = nc.dram_tensor("ident", [128, 128], bf16, kind="ExternalInput").ap()
    out_d = nc.dram_tensor("out", [s_loc, Q, C], f32, kind="ExternalOutput").ap()

    with tile.TileContext(nc) as tc:
        with (
            tc.tile_pool(name="const", bufs=1) as cp,
            tc.tile_pool(name="work", bufs=2) as wp,
            tc.tile_pool(name="work4", bufs=4) as wp4,
            tc.tile_pool(name="pp", bufs=2, space="PSUM") as pp,
            tc.tile_pool(name="sca", bufs=2, space="PSUM") as pscA,
        ):
            # ---- resident constants ----
            wq_t = cp.tile([128, 512], f16, tag="wq")
            wk_t = cp.tile([128, 512], f16, tag="wk")
            wv_t = cp.tile([128, 512], f16, tag="wv")
            wg_t = cp.tile([128, 512], f16, tag="wg")
            wo_t = cp.tile([128, 512], f16, tag="wo")
            expb_t = cp.tile([128, 4096], bf16, tag="expb")
            em_t = cp.tile([128, s_loc * 64], bf16, tag="em")
            bgc_t = cp.tile([128, 2], f32, tag="bgc")
            id_t = cp.tile([128, 128], bf16, tag="ident")

            nc.sync.dma_start(wq_t[:, :], wq_d[:, :])
            nc.sync.dma_start(wk_t[:, :], wk_d[:, :])
            nc.sync.dma_start(wv_t[:, :], wv_d[:, :])
            nc.sync.dma_start(wg_t[:, :], wg_d[:, :])
            nc.sync.dma_start(wo_t[:, :], wo_d[:, :])
            nc.sync.dma_start(expb_t[:, :], expb_d[:, :])
            nc.sync.dma_start(em_t[:, :], em_d[:, :])
            nc.sync.dma_start(bgc_t[:, :], bgc_d[:, :])
            nc.sync.dma_start(id_t[:, :], id_d[:, :])

            # per-pair tiles passed from the load/proj stage to attention
            stash = {}

            def emit_load_proj(p):
                xx = wp.tile([128, 2048], f16, tag="xx")
                for s01 in range(2):
                    nc.sync.dma_start(
                        xx[:, s01 * 1024:(s01 + 1) * 1024].rearrange(
                            "p (cc q) -> p cc q", cc=4),
                        x_d[2 * p + s01].rearrange("(cc p) q -> p cc q", p=128))
                x4 = xx.rearrange("p (s cc q) -> p s cc q", s=2, cc=4)

                # paired projections: rhs = [x_s0 | x_s1] per c-chunk, N=512
                def proj_T(w_t, bcc):
                    ps = pp.tile([128, 1024], f32, tag="pp")
                    for tcc in range(2):
                        for cc in range(2):
                            nc.tensor.matmul(
                                ps[:, tcc * 512:(tcc + 1) * 512].rearrange(
                                    "p (s q) -> p s q", s=2),
                                w_t[:, cc * 256 + tcc * 128:
                                    cc * 256 + tcc * 128 + 128],
                                x4[:, :, bcc + cc, :],
                                start=(cc == 0), stop=(cc == 1))
                    return ps

                qt_ps = proj_T(wq_t, 0)
                qt = wp.tile([128, 1024], f16, tag="qt")
                nc.vector.tensor_copy(qt[:, :], qt_ps[:, :])

                kt_ps = proj_T(wk_t, 2)
                kt = wp.tile([128, 1024], f16, tag="kt")
                nc.vector.tensor_copy(kt[:, :], kt_ps[:, :])

                g_ps = proj_T(wg_t, 0)
                gs0 = wp.tile([128, 1024], f16, tag="gs0")
                for tcc in range(2):
                    nc.scalar.activation(
                        gs0[:, tcc * 512:(tcc + 1) * 512],
                        g_ps[:, tcc * 512:(tcc + 1) * 512],
                        AF.Tanh, scale=0.5,
                        bias=bgc_t[:, tcc:tcc + 1])
                # gs = tanh(.)+1 so the gate apply is a plain 2-input mult
                gs = wp.tile([128, 1024], f16, tag="gs")
                nc.vector.tensor_scalar(
                    gs[:, :], gs0[:, :], 1.0, None, op0=ALU.add)

                # v natural per row; v' = v * exp(mask)[k]
                v_ps = pp.tile([128, 1024], f32, tag="pp")
                for s01 in range(2):
                    for kc in range(2):
                        for cc in range(2):
                            nc.tensor.matmul(
                                v_ps[:, s01 * 512 + kc * 256:
                                     s01 * 512 + kc * 256 + 256],
                                xx[:, s01 * 1024 + 512 + cc * 256 + kc * 128:
                                   s01 * 1024 + 512 + cc * 256 + kc * 128 + 128],
                                wv_t[:, cc * 256:(cc + 1) * 256],
                                start=(cc == 0), stop=(cc == 1))
                vs = wp.tile([128, 1024], bf16, tag="vs")
                for s01 in range(2):
                    s = 2 * p + s01
                    nc.vector.scalar_tensor_tensor(
                        vs[:, s01 * 512:(s01 + 1) * 512].rearrange(
                            "p (kc t) -> p kc t", kc=2),
                        v_ps[:, s01 * 512:(s01 + 1) * 512].rearrange(
                            "p (kc t) -> p kc t", kc=2), 1.0,
                        em_t[:, s * 64:(s + 1) * 64].rearrange(
                            "p (kc e) -> p kc e", kc=2)[:, :, 0:1
                            ].broadcast_to((128, 2, 256)),
                        op0=ALU.mult, op1=ALU.mult)
                stash[p] = (qt, kt, gs, vs)

            def emit_attention(p):
                qt, kt, gs, vs = stash.pop(p)
                ogs = []
                for s01 in range(2):
                    s = 2 * p + s01
                    og = wp4.tile([128, 512], f16, tag="og")
                    ogs.append(og)
                    ovz = pp.tile([128, 1024], f32, tag="pp")
                    for hg in range(2):
                        # per hh-pair: seed pair bias, 2-way packed scores,
                        # exp -> A.  Double-buffered sc pool decouples PE
                        # from ACT (keeps the PE HAM-warm).
                        Axs = []
                        for pr in range(2):
                            sct = pscA.tile([128, 1024], f32, tag="scA")
                            for b in range(2):
                                nc.tensor.matmul(
                                    sct[:, b * 512:(b + 1) * 512],
                                    id_t[:, :],
                                    expb_t[:, hg * 2048 + pr * 1024 + b * 512:
                                           hg * 2048 + pr * 1024 + b * 512 + 512],
                                    start=True, stop=False,
                                    skip_group_check=True)
                            for kc in range(2):
                                for u in range(2):
                                    hh = pr * 2 + u
                                    col = u * 512 + kc * 256
                                    nc.tensor.matmul(
                                        sct[:, col:col + 256],
                                        kt[32 * hh:32 * hh + 32,
                                           hg * 512 + s01 * 256 + kc * 128:
                                           hg * 512 + s01 * 256 + kc * 128 + 128],
                                        qt[32 * hh:32 * hh + 32,
                                           hg * 512 + s01 * 256:
                                           hg * 512 + s01 * 256 + 256],
                                        start=False, stop=(kc == 1),
                                        tile_position=(32 * hh, 0),
                                        skip_group_check=True)
                            Ax = wp4.tile([128, 1024], bf16, tag="A")
                            nc.scalar.activation(Ax[:, :], sct[:, :], AF.Exp)
                            Axs.append(Ax)
                        Aa, Ab = Axs

                        for hh in range(4):
                            Ax = Aa if hh < 2 else Ab
                            for kc in range(2):
                                nc.tensor.matmul(
                                    ovz[32 * hh:32 * hh + 32,
                                        hg * 512:hg * 512 + 256],
                                    vs[:, s01 * 512 + kc * 256 +
                                       (hg * 4 + hh) * 32:
                                       s01 * 512 + kc * 256 +
                                       (hg * 4 + hh) * 32 + 32],
                                    Ax[:, (hh % 2) * 512 + kc * 256:
                                       (hh % 2) * 512 + kc * 256 + 256],
                                    start=(kc == 0), stop=(kc == 1),
                                    tile_position=(0, 32 * hh))
                        for hh in range(4):
                            Ax = Aa if hh < 2 else Ab
                            for kc in range(2):
                                nc.tensor.matmul(
                                    ovz[32 * hh:32 * hh + 32,
                                        hg * 512 + 256:hg * 512 + 512],
                                    em_t[:, s * 64 + kc * 32:
                                         s * 64 + kc * 32 + 32],
                                    Ax[:, (hh % 2) * 512 + kc * 256:
                                       (hh % 2) * 512 + kc * 256 + 256],
                                    start=(kc == 0), stop=(kc == 1),
                                    tile_position=(0, 32 * hh))

                        dd = wp4.tile([128, 256], f16, tag="dd")
                        if use_div:
                            nc.vector.tensor_tensor(
                                dd[:, :], ovz[:, hg * 512:hg * 512 + 256],
                                ovz[:, hg * 512 + 256:hg * 512 + 512],
                                op=ALU.divide)
                        else:
                            rz = wp4.tile([128, 256], f32, tag="rz")
                            nc.vector.reciprocal_approx_fast(
                                rz[:, :], ovz[:, hg * 512 + 256:hg * 512 + 512])
                            nc.vector.tensor_tensor(
                                dd[:, :], ovz[:, hg * 512:hg * 512 + 256],
                                rz[:, :], op=ALU.mult)
                        eng_og = nc.gpsimd if gp_mul else nc.vector
                        eng_og.tensor_tensor(
                            og[:, hg * 256:(hg + 1) * 256],
                            gs[:, hg * 512 + s01 * 256:
                               hg * 512 + s01 * 256 + 256],
                            dd[:, :], op=ALU.mult)

                # final projection for both rows after all scA uses
                f_ps = pp.tile([128, 1024], f32, tag="pp", name="f_ps")
                out_sb = wp.tile([128, 1024], f32, tag="out")
                for s01 in range(2):
                    og = ogs[s01]
                    for qc in range(2):
                        for tcc in range(2):
                            nc.tensor.matmul(
                                f_ps[:, s01 * 512 + qc * 256:
                                     s01 * 512 + qc * 256 + 256],
                                og[:, tcc * 256 + qc * 128:
                                   tcc * 256 + qc * 128 + 128],
                                wo_t[:, tcc * 256:(tcc + 1) * 256],
                                start=(tcc == 0), stop=(tcc == 1))
                # evict the whole pair in one DVE op (bo added on host)
                nc.vector.tensor_copy(out_sb[:, :], f_ps[:, :])
                for s01 in range(2):
                    nc.sync.dma_start(
                        out_d[2 * p + s01].rearrange("(qc p) c -> p qc c", p=128),
                        out_sb[:, s01 * 512:(s01 + 1) * 512].rearrange(
                            "p (qc c) -> p qc c", qc=2))

            # software pipeline: proj(p) emitted before attention(p-1)
            for p in range(npair + 1):
                if p < npair:
                    emit_load_proj(p)
                if p >= 1:
                    emit_attention(p - 1)

    nc.compile()
    return nc


def get_program(s_loc=S_LOC):
    key = (s_loc, os.environ.get("KDIV", "recip"), os.environ.get("KGP", "1"))
    if key not in _CACHE:
        _CACHE[key] = _build_program(s_loc)
    return _CACHE[key]


def prep_inputs(q_x, kv_x, bias_mask, bias_pair, Wq, Wk, Wv, Wg, bg, Wo, bo,
                s_loc=S_LOC, n_cores=N_CORES):
    """Host-side layout prep. Returns per-core in_maps."""
    bf16 = ml_dtypes.bfloat16
    f16 = np.float16

    def wprep(wt):  # (in_dim, out_dim) -> [p, (cc, out)]
        return np.ascontiguousarray(
            wt.reshape(2, 128, 256).transpose(1, 0, 2).reshape(128, 512)
        ).astype(f16)

    wq_h = wprep(np.asarray(Wq).T)     # lhsT[c, t] = Wq[t, c]
    wk_h = wprep(np.asarray(Wk).T)
    wg_h = wprep(np.asarray(Wg).T)
    wv_h = wprep(np.asarray(Wv).T)     # rhs[c, t]
    wo_h = wprep(np.asarray(Wo).T * 0.5)  # rhs[t, c]; 0.5 = sigmoid fold

    bgc = np.ascontiguousarray(
        (0.5 * np.asarray(bg, np.float32)).reshape(2, 128).T)  # [128, tc]

    # pair^T as [128, (hg, pr, u, kc, q)], h = hg*4 + pr*2 + u
    eb = np.asarray(bias_pair[0, 0], np.float64)  # (H, Q, K)
    ebT = eb.transpose(0, 2, 1)  # (H, K, Q)
    expb_h = np.ascontiguousarray(
        ebT.reshape(2, 2, 2, 2, 128, Q).transpose(4, 0, 1, 2, 3, 5
                                                  ).reshape(128, 4096)
    ).astype(bf16)

    x_all = np.concatenate([
        np.asarray(q_x[0], f16).transpose(0, 2, 1),
        np.asarray(kv_x[0], f16).transpose(0, 2, 1)], axis=1)
    x_all = np.ascontiguousarray(x_all)   # (S, 2C, Q): xq | xkv
    # exp(mask) replicated 32x: [128, (s, kc, 32)]
    em_all = np.exp(np.asarray(bias_mask[0, :, 0, 0, :], np.float64))  # (S, K)

    in_maps = []
    for core in range(n_cores):
        lo = core * s_loc
        em = em_all[lo:lo + s_loc].reshape(s_loc, 2, 128)  # (s, kc, p)
        em_h = np.ascontiguousarray(np.broadcast_to(
            em.transpose(2, 0, 1)[:, :, :, None], (128, s_loc, 2, 32)
        ).reshape(128, s_loc * 64)).astype(bf16)
        in_maps.append({
            "x": x_all[lo:lo + s_loc],
            "em": em_h, "expb": expb_h,
            "wq": wq_h, "wk": wk_h, "wv": wv_h, "wg": wg_h, "wo": wo_h,
            "bgc": bgc, "ident": np.eye(128, dtype=bf16),
        })
    return in_maps


def kernel(q_x, kv_x, bias_mask, bias_pair, Wq, Wk, Wv, Wg, bg, Wo, bo):
    from concourse import bass_utils

    nc = get_program()
    in_maps = prep_inputs(q_x, kv_x, bias_mask, bias_pair,
                          Wq, Wk, Wv, Wg, bg, Wo, bo)
    res = bass_utils.run_bass_kernel_spmd(
        nc, in_maps, core_ids=list(range(N_CORES)))
    out = np.concatenate([res.results[i]["out"] for i in range(N_CORES)], axis=0)
    out = out.reshape(B, S, Q, C).astype(np.float32)
    return out + np.asarray(bo, np.float32)
